# revision 1
# baseline (speedup 1.0000x reference)
"""Co-Attention kernel for Trainium2, 8-core SPMD.

Sharding: spatial (H rows) across 8 cores; 32 rows/core with 1-row halo.
Per-core pipeline (all fused, single launch):
  - load input strips into a guard-padded SBUF layout (258-pitch rows)
  - conv1x1+dwconv3x3 folded: 9 PSUM-accumulated matmuls with shifted APs
    (W3_t[o,c] = W1[o,c] * wdw[o,t]) for each of 5 output units
    (q, k_prev, v_prev, k_next, v_next)
  - q/k: PE transpose -> bf16 [n,c] tiles -> Gram matrices (q@kT, self-Grams
    for L2 norms) accumulated on PE over the core's spatial shard
  - v: v_prev+v_next accumulated into an SBUF-resident strip
  - AllReduce of the tiny Gram/norm stats across the 8 cores
  - on-chip double softmax (block-diagonal channel attention)
  - output = (w_proj @ blockdiag(attn_co)) @ v_sum, one matmul per chunk
"""

import sys

sys.path.insert(0, "/opt/trn_rl_repo")

import numpy as np

import concourse.bacc as bacc
import concourse.bass as bass
import concourse.tile as tile
from concourse import mybir
from concourse.bass_utils import run_bass_kernel_spmd

# problem constants
B, C, H, W = 2, 96, 256, 256
HEADS = 4
CH = C // HEADS
N_CORES = 8
RPC = H // N_CORES          # rows per core (32)
SROWS = RPC + 2             # strip rows incl halo (34)
PITCH = W + 2               # guarded row pitch (258)
LEAD = 2                    # leading guard pad
XLEN = LEAD + SROWS * PITCH + 2  # strip flat length (8776)
NTILES = RPC * 2            # 128-wide transpose tiles per unit per b (64)
VLEN = RPC * PITCH          # v_sum flat length per b (8256)

F32 = mybir.dt.float32
BF16 = mybir.dt.bfloat16

# tap offsets (cross-correlation, matching jax.lax.conv_general_dilated)
TAPS = [(ky - 1) * PITCH + (kx - 1) for ky in range(3) for kx in range(3)]

_CACHE = {}


def rowoff(r):
    return LEAD + r * PITCH


def build_kernel():
    import os as _os
    N_UNITS = int(_os.environ.get("N_UNITS", "5"))
    N_B = int(_os.environ.get("N_B", str(B)))
    N_CHUNK = int(_os.environ.get("N_CHUNK", str(RPC // 2)))
    SKIP_SM = bool(_os.environ.get("SKIP_SM"))
    nc = bacc.Bacc("TRN2", target_bir_lowering=False, debug=False,
                   num_devices=N_CORES)

    xc = nc.declare_dram_parameter("xc", [B, C, SROWS, W], F32, isOutput=False)
    xp = nc.declare_dram_parameter("xp", [B, C, SROWS, W], F32, isOutput=False)
    xn = nc.declare_dram_parameter("xn", [B, C, SROWS, W], F32, isOutput=False)
    w3 = nc.declare_dram_parameter("w3", [C, 45, C], F32, isOutput=False)
    wpt = nc.declare_dram_parameter("wpt", [C, C], F32, isOutput=False)
    tmp = nc.declare_dram_parameter("tmp", [C, 1], F32, isOutput=False)
    idn = nc.declare_dram_parameter("idn", [C, C], F32, isOutput=False)
    hmk = nc.declare_dram_parameter("hmk", [C, HEADS], F32, isOutput=False)
    bmk = nc.declare_dram_parameter("bmk", [C, C], F32, isOutput=False)
    y = nc.declare_dram_parameter("y", [B, C, RPC, W], F32, isOutput=True)

    ar_in = nc.dram_tensor("ar_in", [C, 2 * 195], F32)
    ar_out = nc.dram_tensor("ar_out", [C, 2 * 195], F32, addr_space="Shared")

    xsrc = {0: xc, 1: xp, 2: xn}

    with tile.TileContext(nc) as tc:
        with (
            tc.tile_pool(name="singles", bufs=1) as singles,
            tc.tile_pool(name="xpool", bufs=2) as xpool,
            tc.tile_pool(name="dwsb", bufs=3) as dwsbp,
            tc.tile_pool(name="store", bufs=1) as storep,
            tc.tile_pool(name="kstore", bufs=2) as kstorep,
            tc.tile_pool(name="small", bufs=4) as smallp,
            tc.tile_pool(name="outp", bufs=3) as outp,
            tc.tile_pool(name="psdw", bufs=2, space="PSUM") as psdw,
            tc.tile_pool(name="pstp", bufs=2, space="PSUM") as pstp,
            tc.tile_pool(name="psg", bufs=1, space="PSUM") as psg,
        ):
            # ---- constants ----
            w3_sb = singles.tile([C, 45, C], F32)
            nc.sync.dma_start(out=w3_sb[:], in_=w3[:, :, :])
            wpt_sb = singles.tile([C, C], F32)
            nc.sync.dma_start(out=wpt_sb[:], in_=wpt[:, :])
            temp_sb = singles.tile([C, 1], F32)
            nc.sync.dma_start(out=temp_sb[:], in_=tmp[:, :])
            ident = singles.tile([C, C], F32)
            nc.sync.dma_start(out=ident[:], in_=idn[:, :])
            ones1 = singles.tile([1, C], F32)
            nc.vector.memset(ones1[:], 1.0)
            hmask = singles.tile([C, HEADS], F32)
            nc.sync.dma_start(out=hmask[:], in_=hmk[:, :])
            bmask = singles.tile([C, C], F32)
            nc.sync.dma_start(out=bmask[:], in_=bmk[:, :])

            # persistent accumulators
            v_sum = singles.tile([C, B, VLEN], BF16)
            ar_sb = singles.tile([C, B, 195], F32)
            gram_sb = singles.tile([C, B, 5, C], F32)
            arr_sb = singles.tile([C, B, 195], F32)
            mct_sb = singles.tile([C, B, C], BF16)

            qstore = storep.tile([128, NTILES, C], BF16)

            # ---------------- pass 1: conv + dw + grams + v_sum ----------
            for b in range(N_B):
                x_sb = {}
                kT_cur = None
                for u in range(N_UNITS):
                    xi = [0, 1, 1, 2, 2][u]
                    if xi not in x_sb:
                        xt = xpool.tile([C, XLEN], F32, tag="xstrip")
                        # zero guards: leading, trailing, per-row guard cols
                        nc.vector.memset(xt[:, 0:LEAD], 0.0)
                        nc.vector.memset(xt[:, XLEN - 2:XLEN], 0.0)
                        gview = xt[:, LEAD:LEAD + SROWS * PITCH].rearrange(
                            "p (r w) -> p r w", w=PITCH)
                        nc.vector.memset(gview[:, :, W:PITCH], 0.0)
                        nc.sync.dma_start(out=gview[:, :, 0:W],
                                          in_=xsrc[xi][b])
                        x_sb[xi] = xt
                    xt = x_sb[xi]

                    if u == 0:
                        ustore = qstore
                    elif u in (1, 3):
                        ustore = kstorep.tile([128, NTILES, C], BF16,
                                              tag="kT")
                        kT_cur = ustore
                    else:
                        ustore = None

                    if u == 0:
                        g_self = psg.tile([C, C], F32, tag="g")
                    elif u in (1, 3):
                        g_self = psg.tile([C, C], F32, tag="g")
                        g_cross = psg.tile([C, C], F32, tag="g2")

                    for j in range(N_CHUNK):
                        dwps = psdw.tile([C, 2, 512], F32, tag="dwps")
                        for t in range(9):
                            for r2 in range(2):
                                r = 1 + 2 * j + r2
                                off = rowoff(r) + TAPS[t]
                                nc.tensor.matmul(
                                    dwps[:, r2, 0:PITCH],
                                    lhsT=w3_sb[:, u * 9 + t, :],
                                    rhs=xt[:, off:off + PITCH],
                                    start=(t == 0), stop=(t == 8),
                                )
                        if u in (0, 1, 3):
                            dwsb = dwsbp.tile([C, 2, PITCH], F32)
                            nc.scalar.copy(out=dwsb[:], in_=dwps[:, :, 0:PITCH])
                            tp = pstp.tile([128, 4, C], F32)
                            for r2 in range(2):
                                for hf in range(2):
                                    nc.tensor.transpose(
                                        tp[:, 2 * r2 + hf, :],
                                        dwsb[:, r2, 128 * hf:128 * hf + 128],
                                        ident[:],
                                    )
                            i0 = 4 * j
                            nc.vector.tensor_copy(
                                out=ustore[:, i0:i0 + 4, :], in_=tp[:])
                            for i in range(i0, i0 + 4):
                                st = (i == 0)
                                sp = (i == 4 * N_CHUNK - 1)
                                if u == 0:
                                    nc.tensor.matmul(
                                        g_self[:], lhsT=qstore[:, i, :],
                                        rhs=qstore[:, i, :],
                                        start=st, stop=sp,
                                        skip_group_check=True)
                                else:
                                    nc.tensor.matmul(
                                        g_cross[:], lhsT=qstore[:, i, :],
                                        rhs=ustore[:, i, :],
                                        start=st, stop=sp,
                                        skip_group_check=True)
                                    nc.tensor.matmul(
                                        g_self[:], lhsT=ustore[:, i, :],
                                        rhs=ustore[:, i, :],
                                        start=st, stop=sp,
                                        skip_group_check=True)
                        else:
                            vslice = v_sum[:, b, :].rearrange(
                                "p (r w) -> p r w", w=PITCH)[:, 2 * j:2 * j + 2, :]
                            if u == 2:
                                nc.scalar.copy(out=vslice,
                                               in_=dwps[:, :, 0:PITCH])
                            else:
                                nc.vector.tensor_add(
                                    out=vslice, in0=dwps[:, :, 0:PITCH],
                                    in1=vslice)
                    # end unit: evacuate gram psums
                    if u == 0:
                        nc.vector.tensor_copy(out=gram_sb[:, b, 0, :],
                                              in_=g_self[:])
                    elif u == 1:
                        nc.vector.tensor_copy(out=gram_sb[:, b, 1, :],
                                              in_=g_cross[:])
                        nc.vector.tensor_copy(out=gram_sb[:, b, 2, :],
                                              in_=g_self[:])
                    elif u == 3:
                        nc.vector.tensor_copy(out=gram_sb[:, b, 3, :],
                                              in_=g_cross[:])
                        nc.vector.tensor_copy(out=gram_sb[:, b, 4, :],
                                              in_=g_self[:])

                if N_UNITS < 5 or SKIP_SM:
                    continue
                # stats: diag extraction via masked reduce
                scr = smallp.tile([C, C], F32, tag="scr")
                for k, slot in enumerate((0, 2, 4)):
                    nc.vector.tensor_mul(out=scr[:],
                                         in0=gram_sb[:, b, slot, :],
                                         in1=ident[:])
                    nc.vector.reduce_sum(out=ar_sb[:, b, 192 + k:193 + k],
                                         in_=scr[:],
                                         axis=mybir.AxisListType.X)
                nc.vector.tensor_copy(out=ar_sb[:, b, 0:96],
                                      in_=gram_sb[:, b, 1, :])
                nc.vector.tensor_copy(out=ar_sb[:, b, 96:192],
                                      in_=gram_sb[:, b, 3, :])

            # ---------------- all-reduce stats ----------------
            import os as _os
            if SKIP_SM:
                pass
            elif _os.environ.get("SKIP_AR"):
                nc.vector.tensor_copy(
                    out=arr_sb[:].rearrange("p a b -> p (a b)"),
                    in_=ar_sb[:].rearrange("p a b -> p (a b)"))
            else:
                nc.sync.dma_start(out=ar_in[:, :],
                                  in_=ar_sb[:].rearrange("p a b -> p (a b)"))
                nc.gpsimd.collective_compute(
                    "AllReduce", mybir.AluOpType.add,
                    replica_groups=[list(range(N_CORES))],
                    ins=[ar_in[:, :]], outs=[ar_out[:, :]],
                )
                nc.sync.dma_start(
                    out=arr_sb[:].rearrange("p a b -> p (a b)"),
                    in_=ar_out[:, :])

            # ---------------- softmax chain ----------------
            for b in range(B if not SKIP_SM else 0):
                rinv = smallp.tile([C, 3], F32, tag="rinv")
                nc.scalar.activation(out=rinv[:], in_=arr_sb[:, b, 192:195],
                                     func=mybir.ActivationFunctionType.Sqrt)
                nc.vector.tensor_scalar_max(out=rinv[:], in0=rinv[:],
                                            scalar1=1e-12)
                nc.vector.reciprocal(out=rinv[:], in_=rinv[:])
                rqt = smallp.tile([C, 1], F32, tag="rqt")
                nc.vector.tensor_mul(out=rqt[:], in0=rinv[:, 0:1],
                                     in1=temp_sb[:])

                ee = smallp.tile([C, 2, C], F32, tag="ee")
                ssum = smallp.tile([C, 2, HEADS], F32, tag="ssum")
                for s in range(2):
                    logits = smallp.tile([C, C], F32, tag="logits")
                    nc.vector.tensor_scalar_mul(
                        out=logits[:], in0=arr_sb[:, b, 96 * s:96 * s + 96],
                        scalar1=rqt[:])
                    # column scale via transpose sandwich:
                    # Lt = L.T ; Lt *= rk (per-partition) ; L = Lt.T
                    lt_ps = psg.tile([C, C], F32, tag="g")
                    nc.tensor.transpose(lt_ps[:], logits[:], ident[:])
                    lts = smallp.tile([C, C], F32, tag="lts")
                    nc.vector.tensor_scalar_mul(out=lts[:], in0=lt_ps[:],
                                                scalar1=rinv[:, 1 + s:2 + s])
                    lt2_ps = psg.tile([C, C], F32, tag="g2")
                    nc.tensor.transpose(lt2_ps[:], lts[:], ident[:])
                    nc.vector.tensor_copy(out=logits[:], in_=lt2_ps[:])
                    nc.scalar.activation(out=ee[:, s, :], in_=logits[:],
                                         func=mybir.ActivationFunctionType.Exp)
                    nc.vector.reduce_sum(
                        out=ssum[:, s, :],
                        in_=ee[:, s, :].rearrange("p (h d) -> p h d", h=HEADS),
                        axis=mybir.AxisListType.X)
                # rpn = 1/(Sp*Sn) per block
                rpn = smallp.tile([C, HEADS], F32, tag="rpn")
                nc.vector.tensor_mul(out=rpn[:], in0=ssum[:, 0, :],
                                     in1=ssum[:, 1, :])
                nc.vector.reciprocal(out=rpn[:], in_=rpn[:])
                # rc[c] = rpn[c, head(c)] via masked reduce
                scrh = smallp.tile([C, HEADS], F32, tag="scrh")
                rc1 = smallp.tile([C, 1], F32, tag="rc1")
                nc.vector.tensor_mul(out=scrh[:], in0=rpn[:], in1=hmask[:])
                nc.vector.reduce_sum(out=rc1[:], in_=scrh[:],
                                     axis=mybir.AxisListType.X)
                pp = smallp.tile([C, C], F32, tag="pp")
                nc.vector.tensor_mul(out=pp[:], in0=ee[:, 0, :],
                                     in1=ee[:, 1, :])
                nc.vector.tensor_scalar_mul(out=pp[:], in0=pp[:],
                                            scalar1=rc1[:])
                e2 = smallp.tile([C, C], F32, tag="e2")
                nc.scalar.activation(out=e2[:], in_=pp[:],
                                     func=mybir.ActivationFunctionType.Exp)
                s2 = smallp.tile([C, HEADS], F32, tag="s2")
                nc.vector.reduce_sum(
                    out=s2[:], in_=e2[:].rearrange("p (h d) -> p h d", h=HEADS),
                    axis=mybir.AxisListType.X)
                nc.vector.reciprocal(out=s2[:], in_=s2[:])
                rc2 = smallp.tile([C, 1], F32, tag="rc2")
                nc.vector.tensor_mul(out=scrh[:], in0=s2[:], in1=hmask[:])
                nc.vector.reduce_sum(out=rc2[:], in_=scrh[:],
                                     axis=mybir.AxisListType.X)
                bd = smallp.tile([C, C], F32, tag="bd")
                nc.vector.tensor_scalar_mul(out=bd[:], in0=e2[:],
                                            scalar1=rc2[:])
                nc.vector.tensor_mul(out=bd[:], in0=bd[:], in1=bmask[:])
                mct_ps = psg.tile([C, C], F32, tag="g2")
                nc.tensor.matmul(mct_ps[:], lhsT=bd[:], rhs=wpt_sb[:],
                                 start=True, stop=True)
                nc.vector.tensor_copy(out=mct_sb[:, b, :], in_=mct_ps[:])

            # ---------------- pass 2: output ----------------
            for b in range(B if not SKIP_SM else 0):
                vview = v_sum[:, b, :].rearrange("p (r w) -> p r w", w=PITCH)
                for j in range(RPC // 2):
                    ops_ = psdw.tile([C, 2, 512], F32, tag="dwps")
                    for r2 in range(2):
                        nc.tensor.matmul(
                            ops_[:, r2, 0:PITCH], lhsT=mct_sb[:, b, :],
                            rhs=vview[:, 2 * j + r2, :], start=True, stop=True)
                    osb = outp.tile([C, 2, PITCH], F32)
                    nc.scalar.copy(out=osb[:], in_=ops_[:, :, 0:PITCH])
                    nc.sync.dma_start(out=y[b, :, 2 * j:2 * j + 2, :],
                                      in_=osb[:, :, 0:W])

    nc.compile()
    return nc


def _prep_inputs(inputs):
    """Build per-core in_maps from full inputs."""
    x_curr = np.asarray(inputs["x_curr"], np.float32)
    x_prev = np.asarray(inputs["x_prev"], np.float32)
    x_next = np.asarray(inputs["x_next"], np.float32)
    w_q = np.asarray(inputs["w_q"], np.float32)
    w_q_dw = np.asarray(inputs["w_q_dw"], np.float32)
    w_kv_prev = np.asarray(inputs["w_kv_prev"], np.float32)
    w_kv_dw_prev = np.asarray(inputs["w_kv_dw_prev"], np.float32)
    w_kv_next = np.asarray(inputs["w_kv_next"], np.float32)
    w_kv_dw_next = np.asarray(inputs["w_kv_dw_next"], np.float32)
    w_proj = np.asarray(inputs["w_proj"], np.float32)
    temperature = np.asarray(inputs["temperature"], np.float32)

    units = [
        (w_q, w_q_dw.reshape(C, 9)),
        (w_kv_prev[0:C], w_kv_dw_prev[0:C].reshape(C, 9)),
        (w_kv_prev[C:2 * C], w_kv_dw_prev[C:2 * C].reshape(C, 9)),
        (w_kv_next[0:C], w_kv_dw_next[0:C].reshape(C, 9)),
        (w_kv_next[C:2 * C], w_kv_dw_next[C:2 * C].reshape(C, 9)),
    ]
    # w3[c, u*9+t, o] = W1_u[o, c] * wdw_u[o, t]
    w3 = np.zeros((C, 45, C), np.float32)
    for u, (w1, wdw) in enumerate(units):
        w3[:, u * 9:(u + 1) * 9, :] = np.einsum("oc,ot->cto", w1, wdw)

    wpt = np.ascontiguousarray(w_proj.T)
    tmpv = np.repeat(temperature.reshape(HEADS), CH).reshape(C, 1)
    tmpv = np.ascontiguousarray(tmpv, np.float32)
    hmk = np.zeros((C, HEADS), np.float32)
    for h in range(HEADS):
        hmk[h * CH:(h + 1) * CH, h] = 1.0
    bmk = np.zeros((C, C), np.float32)
    for h in range(HEADS):
        bmk[h * CH:(h + 1) * CH, h * CH:(h + 1) * CH] = 1.0

    def strip(x, c):
        r0 = c * RPC - 1
        r1 = c * RPC + RPC + 1
        out = np.zeros((B, C, SROWS, W), np.float32)
        lo, hi = max(r0, 0), min(r1, H)
        out[:, :, lo - r0:lo - r0 + hi - lo, :] = x[:, :, lo:hi, :]
        return out

    in_maps = []
    for c in range(N_CORES):
        in_maps.append({
            "xc": strip(x_curr, c),
            "xp": strip(x_prev, c),
            "xn": strip(x_next, c),
            "w3": w3,
            "wpt": wpt,
            "tmp": tmpv,
            "idn": np.eye(C, dtype=np.float32),
            "hmk": hmk,
            "bmk": bmk,
        })
    return in_maps


def kernel(**inputs):
    if "nc" not in _CACHE:
        _CACHE["nc"] = build_kernel()
    nc = _CACHE["nc"]
    in_maps = _prep_inputs(inputs)
    res = run_bass_kernel_spmd(nc, in_maps, core_ids=list(range(N_CORES)))
    out = np.empty((B, C, H, W), np.float32)
    for c in range(N_CORES):
        out[:, :, c * RPC:(c + 1) * RPC, :] = res.results[c]["y"]
    return out


if __name__ == "__main__":
    rng = np.random.default_rng(0)
    inputs = {
        "x_curr": rng.standard_normal((B, C, H, W), np.float32),
        "x_prev": rng.standard_normal((B, C, H, W), np.float32),
        "x_next": rng.standard_normal((B, C, H, W), np.float32),
        "w_q": rng.standard_normal((C, C), np.float32) * 0.02,
        "w_q_dw": rng.standard_normal((C, 1, 3, 3), np.float32) * 0.02,
        "w_kv_prev": rng.standard_normal((2 * C, C), np.float32) * 0.02,
        "w_kv_dw_prev": rng.standard_normal((2 * C, 1, 3, 3), np.float32) * 0.02,
        "w_kv_next": rng.standard_normal((2 * C, C), np.float32) * 0.02,
        "w_kv_dw_next": rng.standard_normal((2 * C, 1, 3, 3), np.float32) * 0.02,
        "w_proj": rng.standard_normal((C, C), np.float32) * 0.02,
        "temperature": np.ones((HEADS, 1, 1), np.float32),
    }
    out = kernel(**inputs)
    print("out", out.shape, out.dtype, np.abs(out).max())



# revision 14
# speedup vs baseline: 2.8865x; 2.8865x over previous
"""Co-Attention kernel for Trainium2, 8-core SPMD.

Sharding: spatial (H rows) across 8 cores; 32 rows/core with 1-row halo.
Per-core pipeline (all fused, single launch):
  - load input strips into a guard-padded SBUF layout (258-pitch rows)
  - conv1x1+dwconv3x3 folded: 9 PSUM-accumulated matmuls with shifted APs
    (W3_t[o,c] = W1[o,c] * wdw[o,t]) for each of 5 output units
    (q, k_prev, v_prev, k_next, v_next)
  - q/k: PE transpose -> bf16 [n,c] tiles -> Gram matrices (q@kT, self-Grams
    for L2 norms) accumulated on PE over the core's spatial shard
  - v: v_prev+v_next accumulated into an SBUF-resident strip
  - AllReduce of the tiny Gram/norm stats across the 8 cores
  - on-chip double softmax (block-diagonal channel attention)
  - output = (w_proj @ blockdiag(attn_co)) @ v_sum, one matmul per chunk
"""

import sys

sys.path.insert(0, "/opt/trn_rl_repo")

import numpy as np

import concourse.bacc as bacc
import concourse.bass as bass
import concourse.tile as tile
from concourse import mybir
from concourse.bass_utils import run_bass_kernel_spmd

# problem constants
B, C, H, W = 2, 96, 256, 256
HEADS = 4
CH = C // HEADS
N_CORES = 8
RPC = H // N_CORES          # rows per core (32)
SROWS = RPC + 2             # strip rows incl halo (34)
PITCH = W + 2               # guarded row pitch (258)
LEAD = 2                    # leading guard pad
XLEN = LEAD + SROWS * PITCH + 2  # strip flat length (8776)
NTILES = RPC * 2            # 128-wide transpose tiles per unit per b (64)
VLEN = RPC * PITCH          # v_sum flat length per b (8256)

F32 = mybir.dt.float32
F32R = mybir.dt.float32r
BF16 = mybir.dt.bfloat16

# tap offsets (cross-correlation, matching jax.lax.conv_general_dilated)
TAPS = [(ky - 1) * PITCH + (kx - 1) for ky in range(3) for kx in range(3)]

_CACHE = {}


def rowoff(r):
    return LEAD + r * PITCH


def build_kernel():
    import os as _os
    N_UNITS = int(_os.environ.get("N_UNITS", "5"))
    N_B = int(_os.environ.get("N_B", str(B)))
    N_CHUNK = int(_os.environ.get("N_CHUNK", str(RPC // 2)))
    SKIP_SM = bool(_os.environ.get("SKIP_SM"))
    nc = bacc.Bacc("TRN2", target_bir_lowering=False, debug=False,
                   num_devices=N_CORES)

    xc = nc.declare_dram_parameter("xc", [B, C, XLEN], F32R, isOutput=False)
    xp = nc.declare_dram_parameter("xp", [B, C, XLEN], F32R, isOutput=False)
    xn = nc.declare_dram_parameter("xn", [B, C, XLEN], F32R, isOutput=False)
    w3 = nc.declare_dram_parameter("w3", [C, 45, C], F32R, isOutput=False)
    wpt = nc.declare_dram_parameter("wpt", [C, C], F32, isOutput=False)
    tmp = nc.declare_dram_parameter("tmp", [C, 1], F32, isOutput=False)
    idn = nc.declare_dram_parameter("idn", [C, C], F32, isOutput=False)
    hmk = nc.declare_dram_parameter("hmk", [C, HEADS], F32, isOutput=False)
    bmk = nc.declare_dram_parameter("bmk", [C, C], F32, isOutput=False)
    y = nc.declare_dram_parameter("y", [B, C, RPC, W], F32, isOutput=True)

    ar_in = nc.dram_tensor("ar_in", [C, 2 * 195], F32)
    ar_out = nc.dram_tensor("ar_out", [C, 2 * 195], F32, addr_space="Shared")

    xsrc = {0: xc, 1: xp, 2: xn}

    with tile.TileContext(nc) as tc:
        with (
            tc.tile_pool(name="singles", bufs=1) as singles,
            tc.tile_pool(name="xpool", bufs=2) as xpool,
            tc.tile_pool(name="dwsb", bufs=3) as dwsbp,
            tc.tile_pool(name="store", bufs=1) as storep,
            tc.tile_pool(name="kstore", bufs=2) as kstorep,
            tc.tile_pool(name="small", bufs=4) as smallp,
            tc.tile_pool(name="outp", bufs=3) as outp,
            tc.tile_pool(name="psdw", bufs=2, space="PSUM") as psdw,
            tc.tile_pool(name="pstp", bufs=2, space="PSUM") as pstp,
            tc.tile_pool(name="psg", bufs=1, space="PSUM") as psg,
        ):
            # ---- constants ----
            w3_sb = singles.tile([C, 45, C], F32R)
            nc.sync.dma_start(out=w3_sb[:], in_=w3[:, :, :])
            wpt_sb = singles.tile([C, C], F32)
            nc.sync.dma_start(out=wpt_sb[:], in_=wpt[:, :])
            temp_sb = singles.tile([C, 1], F32)
            nc.sync.dma_start(out=temp_sb[:], in_=tmp[:, :])
            ident = singles.tile([C, C], F32)
            nc.sync.dma_start(out=ident[:], in_=idn[:, :])
            ones1 = singles.tile([1, C], F32)
            nc.vector.memset(ones1[:], 1.0)
            hmask = singles.tile([C, HEADS], F32)
            nc.sync.dma_start(out=hmask[:], in_=hmk[:, :])
            bmask = singles.tile([C, C], F32)
            nc.sync.dma_start(out=bmask[:], in_=bmk[:, :])

            # persistent accumulators
            v_sum = singles.tile([C, B, VLEN], BF16)
            ar_sb = singles.tile([C, B, 195], F32)
            gram_sb = singles.tile([C, B, 5, C], F32)
            arr_sb = singles.tile([C, B, 195], F32)
            mct_sb = singles.tile([C, B, C], BF16)

            qstore = storep.tile([128, NTILES, C], BF16)

            # ---------------- pass 1: conv + dw + grams + v_sum ----------
            for b in range(N_B):
                x_sb = {}
                kT_cur = None
                for u in range(N_UNITS):
                    xi = [0, 1, 1, 2, 2][u]
                    if xi not in x_sb:
                        # strip arrives guard-padded from the host; a single
                        # contiguous fp32r DMA (no memsets: fp32r matmul
                        # operands must be produced by fp32r instructions)
                        xt = xpool.tile([C, XLEN], F32R, tag="xstrip")
                        nc.sync.dma_start(out=xt[:], in_=xsrc[xi][b])
                        x_sb[xi] = xt
                    xt = x_sb[xi]

                    if u == 0:
                        ustore = qstore
                    elif u in (1, 3):
                        ustore = kstorep.tile([128, NTILES, C], BF16,
                                              tag="kT")
                        kT_cur = ustore
                    else:
                        ustore = None

                    if u == 0:
                        g_self = psg.tile([C, C], F32, tag="g")
                    elif u in (1, 3):
                        g_self = psg.tile([C, C], F32, tag="g")
                        g_cross = psg.tile([C, C], F32, tag="g2")

                    for j in range(N_CHUNK):
                        dwps = psdw.tile([C, 2, 512], F32, tag="dwps")
                        for t in range(9):
                            for r2 in range(2):
                                r = 1 + 2 * j + r2
                                off = rowoff(r) + TAPS[t]
                                nc.tensor.matmul(
                                    dwps[:, r2, 0:PITCH],
                                    lhsT=w3_sb[:, u * 9 + t, :],
                                    rhs=xt[:, off:off + PITCH],
                                    start=(t == 0), stop=(t == 8),
                                )
                        if u in (0, 1, 3):
                            dwsb = dwsbp.tile([C, 2, PITCH], F32)
                            nc.scalar.copy(out=dwsb[:], in_=dwps[:, :, 0:PITCH])
                            tp = pstp.tile([128, 4, C], F32)
                            for r2 in range(2):
                                for hf in range(2):
                                    nc.tensor.transpose(
                                        tp[:, 2 * r2 + hf, :],
                                        dwsb[:, r2, 128 * hf:128 * hf + 128],
                                        ident[:],
                                    )
                            i0 = 4 * j
                            nc.vector.tensor_copy(
                                out=ustore[:, i0:i0 + 4, :], in_=tp[:])
                            for i in range(i0, i0 + 4):
                                st = (i == 0)
                                sp = (i == 4 * N_CHUNK - 1)
                                if u == 0:
                                    nc.tensor.matmul(
                                        g_self[:], lhsT=qstore[:, i, :],
                                        rhs=qstore[:, i, :],
                                        start=st, stop=sp,
                                        skip_group_check=True)
                                else:
                                    nc.tensor.matmul(
                                        g_cross[:], lhsT=qstore[:, i, :],
                                        rhs=ustore[:, i, :],
                                        start=st, stop=sp,
                                        skip_group_check=True)
                                    nc.tensor.matmul(
                                        g_self[:], lhsT=ustore[:, i, :],
                                        rhs=ustore[:, i, :],
                                        start=st, stop=sp,
                                        skip_group_check=True)
                        else:
                            vslice = v_sum[:, b, :].rearrange(
                                "p (r w) -> p r w", w=PITCH)[:, 2 * j:2 * j + 2, :]
                            if u == 2:
                                nc.scalar.copy(out=vslice,
                                               in_=dwps[:, :, 0:PITCH])
                            else:
                                nc.vector.tensor_add(
                                    out=vslice, in0=dwps[:, :, 0:PITCH],
                                    in1=vslice)
                    # end unit: evacuate gram psums
                    if u == 0:
                        nc.vector.tensor_copy(out=gram_sb[:, b, 0, :],
                                              in_=g_self[:])
                    elif u == 1:
                        nc.vector.tensor_copy(out=gram_sb[:, b, 1, :],
                                              in_=g_cross[:])
                        nc.vector.tensor_copy(out=gram_sb[:, b, 2, :],
                                              in_=g_self[:])
                    elif u == 3:
                        nc.vector.tensor_copy(out=gram_sb[:, b, 3, :],
                                              in_=g_cross[:])
                        nc.vector.tensor_copy(out=gram_sb[:, b, 4, :],
                                              in_=g_self[:])

                if N_UNITS < 5 or SKIP_SM:
                    continue
                # stats: diag extraction via masked reduce
                scr = smallp.tile([C, C], F32, tag="scr")
                for k, slot in enumerate((0, 2, 4)):
                    nc.vector.tensor_mul(out=scr[:],
                                         in0=gram_sb[:, b, slot, :],
                                         in1=ident[:])
                    nc.vector.reduce_sum(out=ar_sb[:, b, 192 + k:193 + k],
                                         in_=scr[:],
                                         axis=mybir.AxisListType.X)
                nc.vector.tensor_copy(out=ar_sb[:, b, 0:96],
                                      in_=gram_sb[:, b, 1, :])
                nc.vector.tensor_copy(out=ar_sb[:, b, 96:192],
                                      in_=gram_sb[:, b, 3, :])

            # ---------------- all-reduce stats ----------------
            import os as _os
            if SKIP_SM:
                pass
            elif _os.environ.get("SKIP_AR"):
                nc.vector.tensor_copy(
                    out=arr_sb[:].rearrange("p a b -> p (a b)"),
                    in_=ar_sb[:].rearrange("p a b -> p (a b)"))
            else:
                nc.sync.dma_start(out=ar_in[:, :],
                                  in_=ar_sb[:].rearrange("p a b -> p (a b)"))
                nc.gpsimd.collective_compute(
                    "AllReduce", mybir.AluOpType.add,
                    replica_groups=[list(range(N_CORES))],
                    ins=[ar_in[:, :]], outs=[ar_out[:, :]],
                )
                nc.sync.dma_start(
                    out=arr_sb[:].rearrange("p a b -> p (a b)"),
                    in_=ar_out[:, :])

            # ---------------- softmax chain ----------------
            for b in range(B if not SKIP_SM else 0):
                rinv = smallp.tile([C, 3], F32, tag="rinv")
                nc.scalar.activation(out=rinv[:], in_=arr_sb[:, b, 192:195],
                                     func=mybir.ActivationFunctionType.Sqrt)
                nc.vector.tensor_scalar_max(out=rinv[:], in0=rinv[:],
                                            scalar1=1e-12)
                nc.vector.reciprocal(out=rinv[:], in_=rinv[:])
                rqt = smallp.tile([C, 1], F32, tag="rqt")
                nc.vector.tensor_mul(out=rqt[:], in0=rinv[:, 0:1],
                                     in1=temp_sb[:])

                ee = smallp.tile([C, 2, C], F32, tag="ee")
                ssum = smallp.tile([C, 2, HEADS], F32, tag="ssum")
                for s in range(2):
                    logits = smallp.tile([C, C], F32, tag="logits")
                    nc.vector.tensor_scalar_mul(
                        out=logits[:], in0=arr_sb[:, b, 96 * s:96 * s + 96],
                        scalar1=rqt[:])
                    # column scale via transpose sandwich:
                    # Lt = L.T ; Lt *= rk (per-partition) ; L = Lt.T
                    lt_ps = psg.tile([C, C], F32, tag="g")
                    nc.tensor.transpose(lt_ps[:], logits[:], ident[:])
                    lts = smallp.tile([C, C], F32, tag="lts")
                    nc.vector.tensor_scalar_mul(out=lts[:], in0=lt_ps[:],
                                                scalar1=rinv[:, 1 + s:2 + s])
                    lt2_ps = psg.tile([C, C], F32, tag="g2")
                    nc.tensor.transpose(lt2_ps[:], lts[:], ident[:])
                    nc.vector.tensor_copy(out=logits[:], in_=lt2_ps[:])
                    nc.scalar.activation(out=ee[:, s, :], in_=logits[:],
                                         func=mybir.ActivationFunctionType.Exp)
                    nc.vector.reduce_sum(
                        out=ssum[:, s, :],
                        in_=ee[:, s, :].rearrange("p (h d) -> p h d", h=HEADS),
                        axis=mybir.AxisListType.X)
                # rpn = 1/(Sp*Sn) per block
                rpn = smallp.tile([C, HEADS], F32, tag="rpn")
                nc.vector.tensor_mul(out=rpn[:], in0=ssum[:, 0, :],
                                     in1=ssum[:, 1, :])
                nc.vector.reciprocal(out=rpn[:], in_=rpn[:])
                # rc[c] = rpn[c, head(c)] via masked reduce
                scrh = smallp.tile([C, HEADS], F32, tag="scrh")
                rc1 = smallp.tile([C, 1], F32, tag="rc1")
                nc.vector.tensor_mul(out=scrh[:], in0=rpn[:], in1=hmask[:])
                nc.vector.reduce_sum(out=rc1[:], in_=scrh[:],
                                     axis=mybir.AxisListType.X)
                pp = smallp.tile([C, C], F32, tag="pp")
                nc.vector.tensor_mul(out=pp[:], in0=ee[:, 0, :],
                                     in1=ee[:, 1, :])
                nc.vector.tensor_scalar_mul(out=pp[:], in0=pp[:],
                                            scalar1=rc1[:])
                e2 = smallp.tile([C, C], F32, tag="e2")
                nc.scalar.activation(out=e2[:], in_=pp[:],
                                     func=mybir.ActivationFunctionType.Exp)
                s2 = smallp.tile([C, HEADS], F32, tag="s2")
                nc.vector.reduce_sum(
                    out=s2[:], in_=e2[:].rearrange("p (h d) -> p h d", h=HEADS),
                    axis=mybir.AxisListType.X)
                nc.vector.reciprocal(out=s2[:], in_=s2[:])
                rc2 = smallp.tile([C, 1], F32, tag="rc2")
                nc.vector.tensor_mul(out=scrh[:], in0=s2[:], in1=hmask[:])
                nc.vector.reduce_sum(out=rc2[:], in_=scrh[:],
                                     axis=mybir.AxisListType.X)
                bd = smallp.tile([C, C], F32, tag="bd")
                nc.vector.tensor_scalar_mul(out=bd[:], in0=e2[:],
                                            scalar1=rc2[:])
                nc.vector.tensor_mul(out=bd[:], in0=bd[:], in1=bmask[:])
                mct_ps = psg.tile([C, C], F32, tag="g2")
                nc.tensor.matmul(mct_ps[:], lhsT=bd[:], rhs=wpt_sb[:],
                                 start=True, stop=True)
                nc.vector.tensor_copy(out=mct_sb[:, b, :], in_=mct_ps[:])

            # ---------------- pass 2: output ----------------
            for b in range(B if not SKIP_SM else 0):
                vview = v_sum[:, b, :].rearrange("p (r w) -> p r w", w=PITCH)
                for j in range(RPC // 2):
                    ops_ = psdw.tile([C, 2, 512], F32, tag="dwps")
                    for r2 in range(2):
                        nc.tensor.matmul(
                            ops_[:, r2, 0:PITCH], lhsT=mct_sb[:, b, :],
                            rhs=vview[:, 2 * j + r2, :], start=True, stop=True)
                    osb = outp.tile([C, 2, PITCH], F32)
                    nc.scalar.copy(out=osb[:], in_=ops_[:, :, 0:PITCH])
                    nc.sync.dma_start(out=y[b, :, 2 * j:2 * j + 2, :],
                                      in_=osb[:, :, 0:W])

    nc.compile()
    return nc


def _prep_inputs(inputs):
    """Build per-core in_maps from full inputs."""
    x_curr = np.asarray(inputs["x_curr"], np.float32)
    x_prev = np.asarray(inputs["x_prev"], np.float32)
    x_next = np.asarray(inputs["x_next"], np.float32)
    w_q = np.asarray(inputs["w_q"], np.float32)
    w_q_dw = np.asarray(inputs["w_q_dw"], np.float32)
    w_kv_prev = np.asarray(inputs["w_kv_prev"], np.float32)
    w_kv_dw_prev = np.asarray(inputs["w_kv_dw_prev"], np.float32)
    w_kv_next = np.asarray(inputs["w_kv_next"], np.float32)
    w_kv_dw_next = np.asarray(inputs["w_kv_dw_next"], np.float32)
    w_proj = np.asarray(inputs["w_proj"], np.float32)
    temperature = np.asarray(inputs["temperature"], np.float32)

    units = [
        (w_q, w_q_dw.reshape(C, 9)),
        (w_kv_prev[0:C], w_kv_dw_prev[0:C].reshape(C, 9)),
        (w_kv_prev[C:2 * C], w_kv_dw_prev[C:2 * C].reshape(C, 9)),
        (w_kv_next[0:C], w_kv_dw_next[0:C].reshape(C, 9)),
        (w_kv_next[C:2 * C], w_kv_dw_next[C:2 * C].reshape(C, 9)),
    ]
    # w3[c, u*9+t, o] = W1_u[o, c] * wdw_u[o, t]
    w3 = np.zeros((C, 45, C), np.float32)
    for u, (w1, wdw) in enumerate(units):
        w3[:, u * 9:(u + 1) * 9, :] = np.einsum("oc,ot->cto", w1, wdw)

    wpt = np.ascontiguousarray(w_proj.T)
    tmpv = np.repeat(temperature.reshape(HEADS), CH).reshape(C, 1)
    tmpv = np.ascontiguousarray(tmpv, np.float32)
    hmk = np.zeros((C, HEADS), np.float32)
    for h in range(HEADS):
        hmk[h * CH:(h + 1) * CH, h] = 1.0
    bmk = np.zeros((C, C), np.float32)
    for h in range(HEADS):
        bmk[h * CH:(h + 1) * CH, h * CH:(h + 1) * CH] = 1.0

    def strip(x, c):
        # guard-padded flat strip: LEAD zeros + SROWS rows of (W data +
        # 2 zero guard cols) + 2 trailing zeros
        r0 = c * RPC - 1
        r1 = c * RPC + RPC + 1
        rows = np.zeros((B, C, SROWS, PITCH), np.float32)
        lo, hi = max(r0, 0), min(r1, H)
        rows[:, :, lo - r0:lo - r0 + hi - lo, 0:W] = x[:, :, lo:hi, :]
        out = np.zeros((B, C, XLEN), np.float32)
        out[:, :, LEAD:LEAD + SROWS * PITCH] = rows.reshape(B, C, -1)
        return out

    in_maps = []
    for c in range(N_CORES):
        in_maps.append({
            "xc": strip(x_curr, c),
            "xp": strip(x_prev, c),
            "xn": strip(x_next, c),
            "w3": w3,
            "wpt": wpt,
            "tmp": tmpv,
            "idn": np.eye(C, dtype=np.float32),
            "hmk": hmk,
            "bmk": bmk,
        })
    return in_maps


def kernel(**inputs):
    if "nc" not in _CACHE:
        _CACHE["nc"] = build_kernel()
    nc = _CACHE["nc"]
    in_maps = _prep_inputs(inputs)
    res = run_bass_kernel_spmd(nc, in_maps, core_ids=list(range(N_CORES)))
    out = np.empty((B, C, H, W), np.float32)
    for c in range(N_CORES):
        out[:, :, c * RPC:(c + 1) * RPC, :] = res.results[c]["y"]
    return out


if __name__ == "__main__":
    rng = np.random.default_rng(0)
    inputs = {
        "x_curr": rng.standard_normal((B, C, H, W), np.float32),
        "x_prev": rng.standard_normal((B, C, H, W), np.float32),
        "x_next": rng.standard_normal((B, C, H, W), np.float32),
        "w_q": rng.standard_normal((C, C), np.float32) * 0.02,
        "w_q_dw": rng.standard_normal((C, 1, 3, 3), np.float32) * 0.02,
        "w_kv_prev": rng.standard_normal((2 * C, C), np.float32) * 0.02,
        "w_kv_dw_prev": rng.standard_normal((2 * C, 1, 3, 3), np.float32) * 0.02,
        "w_kv_next": rng.standard_normal((2 * C, C), np.float32) * 0.02,
        "w_kv_dw_next": rng.standard_normal((2 * C, 1, 3, 3), np.float32) * 0.02,
        "w_proj": rng.standard_normal((C, C), np.float32) * 0.02,
        "temperature": np.ones((HEADS, 1, 1), np.float32),
    }
    out = kernel(**inputs)
    print("out", out.shape, out.dtype, np.abs(out).max())



# revision 20
# speedup vs baseline: 4.4817x; 1.5526x over previous
"""Co-Attention kernel for Trainium2, 8-core SPMD.

Sharding: spatial (H rows) across 8 cores; 32 rows/core with 1-row halo.
Per-core pipeline (all fused, single launch):
  - conv1x1+dwconv3x3 folded into shifted matmuls with combined weights
    W3_t[o,c] = W1[o,c] * wdw[o,t]; operands quantized to fp8e4 and run in
    DoubleRow perf mode with TWO taps packed per matmul via the k-tile dim
    (rhs dim1 = tap-pair with a strided/overlapping access pattern)
  - q/k: PSUM -> bf16 -> PE transpose -> [n,c] tiles -> Gram matrices
    (q@kT + self-Grams for the L2 norms) accumulated on PE over the
    core's spatial shard; v_prev+v_next accumulate into one PSUM chain
    and land in a bf16 SBUF-resident strip
  - per-batch AllReduce of the tiny Gram/norm stats across the 8 cores,
    software-pipelined against the other batch's conv pass
  - on-chip double softmax (block-diagonal channel attention)
  - output = (w_proj @ blockdiag(attn_co)) @ v_sum, one matmul per 2 rows
"""

import os
import sys

sys.path.insert(0, "/opt/trn_rl_repo")

import numpy as np
import ml_dtypes

import concourse.bacc as bacc
import concourse.bass as bass
import concourse.tile as tile
from concourse import mybir
from concourse.ap import AP
from concourse.bass_utils import run_bass_kernel_spmd

# problem constants
B, C, H, W = 2, 96, 256, 256
HEADS = 4
CH = C // HEADS
N_CORES = 8
RPC = H // N_CORES          # rows per core (32)
SROWS = RPC + 2             # strip rows incl halo (34)
PITCH = W + 2               # guarded row pitch (258)
LEAD = 2                    # leading guard pad
XLEN = LEAD + SROWS * PITCH + 2  # strip flat length (8776)
NCH = RPC // 2              # 2-row chunks per core (16)
NTILES = RPC * 2            # 128-wide transpose tiles per unit per b (64)
VLEN = RPC * W              # v_sum flat length per b (8192)
WSCALE = 4096.0             # fp8 weight scale (exact power of 2)

F32 = mybir.dt.float32
BF16 = mybir.dt.bfloat16
F8 = mybir.dt.float8e4
DR = mybir.MatmulPerfMode.DoubleRow

# tap offsets (cross-correlation, matching jax.lax.conv_general_dilated)
TAPS = [(ky - 1) * PITCH + (kx - 1) for ky in range(3) for kx in range(3)]
# tap-pair packing for DoubleRow: (first tap, second tap, rhs dim1 stride);
# last pair duplicates tap 8 with zero weights (stride 0)
PAIR_T = [(0, 1, 1), (2, 5, PITCH), (3, 4, 1), (6, 7, 1), (8, None, 0)]

_CACHE = {}


def rowoff(r):
    return LEAD + r * PITCH


def pair_rhs(xt, j, m):
    """Overlapping strided rhs AP [C, ktile=2, rows=2, cols=256] for the
    DoubleRow tap-pair matmul of chunk j, pair m."""
    t0, _, dstride = PAIR_T[m]
    base = xt[:]
    off = base.offset + rowoff(1 + 2 * j) + TAPS[t0]
    ppair = list(base.ap[0])
    return AP(base.tensor, off,
              [ppair, [dstride, 2], [PITCH, 2], [1, W]])


def row2_rhs(xt, j, t):
    """Plain 2-row rhs AP [C, rows=2, cols=256] for tap t of chunk j."""
    base = xt[:]
    off = base.offset + rowoff(1 + 2 * j) + TAPS[t]
    ppair = list(base.ap[0])
    return AP(base.tensor, off, [ppair, [PITCH, 2], [1, W]])


def build_kernel():
    SKIP_AR = bool(os.environ.get("SKIP_AR"))
    nc = bacc.Bacc("TRN2", target_bir_lowering=False, debug=False,
                   num_devices=N_CORES)

    x8c = nc.declare_dram_parameter("x8c", [B, C, XLEN], F8, isOutput=False)
    x8p = nc.declare_dram_parameter("x8p", [B, C, XLEN], F8, isOutput=False)
    x8n = nc.declare_dram_parameter("x8n", [B, C, XLEN], F8, isOutput=False)
    w3qk = nc.declare_dram_parameter("w3qk", [C, 3, 10, C], F8, isOutput=False)
    xbp = nc.declare_dram_parameter("xbp", [B, C, XLEN], BF16, isOutput=False)
    xbn = nc.declare_dram_parameter("xbn", [B, C, XLEN], BF16, isOutput=False)
    w3v = nc.declare_dram_parameter("w3v", [C, 2, 9, C], BF16, isOutput=False)
    wpt = nc.declare_dram_parameter("wpt", [C, C], F32, isOutput=False)
    tmp = nc.declare_dram_parameter("tmp", [C, 1], F32, isOutput=False)
    idn = nc.declare_dram_parameter("idn", [C, C], F32, isOutput=False)
    hmk = nc.declare_dram_parameter("hmk", [C, HEADS], F32, isOutput=False)
    bmk = nc.declare_dram_parameter("bmk", [C, C], F32, isOutput=False)
    y = nc.declare_dram_parameter("y", [B, C, RPC, W], F32, isOutput=True)

    ar_in = [nc.dram_tensor(f"ar_in{b}", [C, 195], F32) for b in range(B)]
    ar_out = [nc.dram_tensor(f"ar_out{b}", [C, 195], F32, addr_space="Shared")
              for b in range(B)]

    with tile.TileContext(nc) as tc:
        with (
            tc.tile_pool(name="singles", bufs=1) as singles,
            tc.tile_pool(name="xpool", bufs=1) as xpool,
            tc.tile_pool(name="dwsb", bufs=3) as dwsbp,
            tc.tile_pool(name="qstorep", bufs=2) as qstorep,
            tc.tile_pool(name="kstore", bufs=3) as kstorep,
            tc.tile_pool(name="small", bufs=2) as smallp,
            tc.tile_pool(name="outp", bufs=2) as outp,
            tc.tile_pool(name="psdw", bufs=2, space="PSUM") as psdw,
            tc.tile_pool(name="pstp", bufs=2, space="PSUM") as pstp,
            tc.tile_pool(name="psg", bufs=1, space="PSUM") as psg,
        ):
            # ---- constants ----
            w3qk_sb = singles.tile([C, 3, 10, C], F8)
            nc.sync.dma_start(out=w3qk_sb[:], in_=w3qk[:, :, :, :])
            w3v_sb = singles.tile([C, 2, 9, C], BF16)
            nc.sync.dma_start(out=w3v_sb[:], in_=w3v[:, :, :, :])
            wpt_sb = singles.tile([C, C], F32)
            nc.sync.dma_start(out=wpt_sb[:], in_=wpt[:, :])
            temp_sb = singles.tile([C, 1], F32)
            nc.sync.dma_start(out=temp_sb[:], in_=tmp[:, :])
            ident = singles.tile([C, C], F32)
            nc.sync.dma_start(out=ident[:], in_=idn[:, :])
            identb = singles.tile([C, C], BF16)
            nc.vector.tensor_copy(out=identb[:], in_=ident[:])
            hmask = singles.tile([C, HEADS], F32)
            nc.sync.dma_start(out=hmask[:], in_=hmk[:, :])
            bmask = singles.tile([C, C], F32)
            nc.sync.dma_start(out=bmask[:], in_=bmk[:, :])

            # input strips: all 6 up-front (no deps, fills SBUF early)
            xs = {}
            for b in range(B):
                for key, src in (("c", x8c), ("p", x8p), ("n", x8n)):
                    pass
                for key, src in (("c", x8c), ("p", x8p), ("n", x8n),
                                 ("bp", xbp), ("bn", xbn)):
                    dt_ = BF16 if key.startswith("b") else F8
                    t = xpool.tile([C, XLEN], dt_, tag=f"x{key}{b}",
                                   name="xstrip")
                    nc.sync.dma_start(out=t[:], in_=src[b])
                    xs[key, b] = t

            # persistent accumulators
            v_sum = singles.tile([C, B, VLEN], BF16)
            ar_sb = singles.tile([C, B, 195], F32)
            gram_sb = singles.tile([C, B, 5, C], F32)
            arr_sb = singles.tile([C, B, 195], F32)
            mct_sb = singles.tile([C, B, C], BF16)

            qstore = {}

            # ------------ pass 1 pieces ------------
            def qk_unit(b, u, xt, is_q):
                """q or k unit: 5-stage chunk pipeline.
                stages: taps(PE) -> evac(Act) -> transpose(PE) ->
                        ucopy(DVE) -> grams(PE)."""
                if is_q:
                    qstore[b] = qstorep.tile([128, NTILES, C], BF16,
                                             tag="qstore", name="qstore")
                    g_self = psg.tile([C, C], F32, tag="g")
                    g_cross = None
                else:
                    g_self = psg.tile([C, C], F32, tag="g")
                    g_cross = psg.tile([C, C], F32, tag="g2")
                dw = [None] * NCH
                sb = [None] * NCH
                tpt = [None] * NCH
                kt = [None] * NCH

                def taps(j):
                    dwps = psdw.tile([C, 2, W], F32, tag="dwps")
                    for m in range(5):
                        nc.tensor.matmul(
                            dwps[:], lhsT=w3qk_sb[:, u, 2 * m:2 * m + 2, :],
                            rhs=pair_rhs(xt, j, m),
                            start=(m == 0), stop=(m == 4), perf_mode=DR)
                    dw[j] = dwps

                def evac(j):
                    dwsb = dwsbp.tile([C, 2, W], BF16)
                    nc.scalar.copy(out=dwsb[:], in_=dw[j][:])
                    sb[j] = dwsb

                def transp(j):
                    tp = pstp.tile([128, 4, C], BF16, tag="tp")
                    for r2 in range(2):
                        for hf in range(2):
                            nc.tensor.transpose(
                                tp[:, 2 * r2 + hf, :],
                                sb[j][:, r2, 128 * hf:128 * hf + 128],
                                identb[:])
                    tpt[j] = tp

                def ucopy(j):
                    if is_q:
                        dst = qstore[b][:, 4 * j:4 * j + 4, :]
                    else:
                        kt[j] = kstorep.tile([128, 4, C], BF16, tag="kT",
                                             name="kt")
                        dst = kt[j][:]
                    nc.vector.tensor_copy(out=dst, in_=tpt[j][:])

                def gram(j):
                    for i in range(4):
                        gi = 4 * j + i
                        st = (gi == 0)
                        sp = (gi == NTILES - 1)
                        if is_q:
                            nc.tensor.matmul(
                                g_self[:], lhsT=qstore[b][:, gi, :],
                                rhs=qstore[b][:, gi, :], start=st, stop=sp,
                                skip_group_check=True)
                        else:
                            nc.tensor.matmul(
                                g_cross[:], lhsT=qstore[b][:, gi, :],
                                rhs=kt[j][:, i, :], start=st, stop=sp,
                                skip_group_check=True)
                            nc.tensor.matmul(
                                g_self[:], lhsT=kt[j][:, i, :],
                                rhs=kt[j][:, i, :], start=st, stop=sp,
                                skip_group_check=True)

                stages = [taps, evac, transp, ucopy, gram]
                for j in range(NCH + 4):
                    for s, fn in enumerate(stages):
                        if 0 <= j - s < NCH:
                            fn(j - s)

                # evacuate gram psums
                if is_q:
                    nc.vector.tensor_copy(out=gram_sb[:, b, 0, :],
                                          in_=g_self[:])
                else:
                    slot = 1 if u == 1 else 3
                    nc.vector.tensor_copy(out=gram_sb[:, b, slot, :],
                                          in_=g_cross[:])
                    nc.vector.tensor_copy(out=gram_sb[:, b, slot + 1, :],
                                          in_=g_self[:])

            def v_unit(b):
                """fused v_prev+v_next: 10 DR tap matmuls into one PSUM
                accumulation per chunk, Pool evacuates to bf16 v_sum."""
                dw = [None] * NCH

                def taps(j):
                    dwps = psdw.tile([C, 2, W], F32, tag="dwps")
                    for part, key in enumerate(("bp", "bn")):
                        for t in range(9):
                            nc.tensor.matmul(
                                dwps[:],
                                lhsT=w3v_sb[:, part, t, :],
                                rhs=row2_rhs(xs[key, b], j, t),
                                start=(part == 0 and t == 0),
                                stop=(part == 1 and t == 8))
                    dw[j] = dwps

                def evac(j):
                    nc.vector.tensor_copy(
                        out=v_sum[:, b, 2 * W * j:2 * W * (j + 1)].rearrange(
                            "p (r w) -> p r w", w=W),
                        in_=dw[j][:])

                for j in range(NCH + 1):
                    if j < NCH:
                        taps(j)
                    if j >= 1:
                        evac(j - 1)

            def stats_and_ar(b):
                # diag extraction via masked reduce + AllReduce kickoff
                scr = smallp.tile([C, C], F32, tag="scr")
                for k, slot in enumerate((0, 2, 4)):
                    nc.vector.tensor_mul(out=scr[:],
                                         in0=gram_sb[:, b, slot, :],
                                         in1=ident[:])
                    nc.vector.reduce_sum(out=ar_sb[:, b, 192 + k:193 + k],
                                         in_=scr[:],
                                         axis=mybir.AxisListType.X)
                nc.vector.tensor_copy(out=ar_sb[:, b, 0:96],
                                      in_=gram_sb[:, b, 1, :])
                nc.vector.tensor_copy(out=ar_sb[:, b, 96:192],
                                      in_=gram_sb[:, b, 3, :])
                if SKIP_AR:
                    nc.vector.tensor_copy(out=arr_sb[:, b, :],
                                          in_=ar_sb[:, b, :])
                else:
                    nc.sync.dma_start(out=ar_in[b][:, :], in_=ar_sb[:, b, :])
                    nc.gpsimd.collective_compute(
                        "AllReduce", mybir.AluOpType.add,
                        replica_groups=[list(range(N_CORES))],
                        ins=[ar_in[b][:, :]], outs=[ar_out[b][:, :]],
                    )
                    nc.sync.dma_start(out=arr_sb[:, b, :],
                                      in_=ar_out[b][:, :])

            def softmax(b):
                rinv = smallp.tile([C, 3], F32, tag="rinv")
                nc.scalar.activation(out=rinv[:], in_=arr_sb[:, b, 192:195],
                                     func=mybir.ActivationFunctionType.Sqrt)
                nc.vector.tensor_scalar_max(out=rinv[:], in0=rinv[:],
                                            scalar1=1e-12)
                nc.vector.reciprocal(out=rinv[:], in_=rinv[:])
                rqt = smallp.tile([C, 1], F32, tag="rqt")
                nc.vector.tensor_mul(out=rqt[:], in0=rinv[:, 0:1],
                                     in1=temp_sb[:])

                ee = smallp.tile([C, 2, C], F32, tag="ee")
                ssum = smallp.tile([C, 2, HEADS], F32, tag="ssum")
                for s in range(2):
                    logits = smallp.tile([C, C], F32, tag="logits")
                    nc.vector.tensor_scalar_mul(
                        out=logits[:], in0=arr_sb[:, b, 96 * s:96 * s + 96],
                        scalar1=rqt[:])
                    # column scale via transpose sandwich
                    lt_ps = psg.tile([C, C], F32, tag="g")
                    nc.tensor.transpose(lt_ps[:], logits[:], ident[:])
                    lts = smallp.tile([C, C], F32, tag="lts")
                    nc.vector.tensor_scalar_mul(out=lts[:], in0=lt_ps[:],
                                                scalar1=rinv[:, 1 + s:2 + s])
                    lt2_ps = psg.tile([C, C], F32, tag="g2")
                    nc.tensor.transpose(lt2_ps[:], lts[:], ident[:])
                    nc.vector.tensor_copy(out=logits[:], in_=lt2_ps[:])
                    nc.scalar.activation(out=ee[:, s, :], in_=logits[:],
                                         func=mybir.ActivationFunctionType.Exp)
                    nc.vector.reduce_sum(
                        out=ssum[:, s, :],
                        in_=ee[:, s, :].rearrange("p (h d) -> p h d", h=HEADS),
                        axis=mybir.AxisListType.X)
                # rpn = 1/(Sp*Sn) per block
                rpn = smallp.tile([C, HEADS], F32, tag="rpn")
                nc.vector.tensor_mul(out=rpn[:], in0=ssum[:, 0, :],
                                     in1=ssum[:, 1, :])
                nc.vector.reciprocal(out=rpn[:], in_=rpn[:])
                scrh = smallp.tile([C, HEADS], F32, tag="scrh")
                rc1 = smallp.tile([C, 1], F32, tag="rc1")
                nc.vector.tensor_mul(out=scrh[:], in0=rpn[:], in1=hmask[:])
                nc.vector.reduce_sum(out=rc1[:], in_=scrh[:],
                                     axis=mybir.AxisListType.X)
                pp = smallp.tile([C, C], F32, tag="pp")
                nc.vector.tensor_mul(out=pp[:], in0=ee[:, 0, :],
                                     in1=ee[:, 1, :])
                nc.vector.tensor_scalar_mul(out=pp[:], in0=pp[:],
                                            scalar1=rc1[:])
                e2 = smallp.tile([C, C], F32, tag="e2")
                nc.scalar.activation(out=e2[:], in_=pp[:],
                                     func=mybir.ActivationFunctionType.Exp)
                s2 = smallp.tile([C, HEADS], F32, tag="s2")
                nc.vector.reduce_sum(
                    out=s2[:],
                    in_=e2[:].rearrange("p (h d) -> p h d", h=HEADS),
                    axis=mybir.AxisListType.X)
                nc.vector.reciprocal(out=s2[:], in_=s2[:])
                rc2 = smallp.tile([C, 1], F32, tag="rc2")
                nc.vector.tensor_mul(out=scrh[:], in0=s2[:], in1=hmask[:])
                nc.vector.reduce_sum(out=rc2[:], in_=scrh[:],
                                     axis=mybir.AxisListType.X)
                bd = smallp.tile([C, C], F32, tag="bd")
                nc.vector.tensor_scalar_mul(out=bd[:], in0=e2[:],
                                            scalar1=rc2[:])
                nc.vector.tensor_mul(out=bd[:], in0=bd[:], in1=bmask[:])
                mct_ps = psg.tile([C, C], F32, tag="g2")
                nc.tensor.matmul(mct_ps[:], lhsT=bd[:], rhs=wpt_sb[:],
                                 start=True, stop=True)
                nc.vector.tensor_copy(out=mct_sb[:, b, :], in_=mct_ps[:])

            def pass2(b):
                ops = [None] * NCH

                def mm(j):
                    t = psdw.tile([C, 2, W], F32, tag="dwps")
                    nc.tensor.matmul(
                        t[:], lhsT=mct_sb[:, b, :],
                        rhs=v_sum[:, b, 2 * W * j:2 * W * (j + 1)],
                        start=True, stop=True)
                    ops[j] = t

                def evac(j):
                    osb = outp.tile([C, 2, W], F32)
                    if j % 2 == 0:
                        nc.scalar.copy(out=osb[:], in_=ops[j][:])
                    else:
                        nc.vector.tensor_copy(out=osb[:], in_=ops[j][:])
                    nc.sync.dma_start(out=y[b, :, 2 * j:2 * j + 2, :],
                                      in_=osb[:])

                for j in range(NCH + 1):
                    if j < NCH:
                        mm(j)
                    if j >= 1:
                        evac(j - 1)

            # ------------ emission schedule ------------
            # b0 conv -> AR(b0) overlaps [v(b0), q(b1)] -> softmax(b0)
            # -> pass2(b0) after kn(b1) -> AR(b1) overlaps [v(b1), pass2(b0)]
            qk_unit(0, 0, xs["c", 0], True)
            qk_unit(0, 1, xs["p", 0], False)
            qk_unit(0, 2, xs["n", 0], False)
            stats_and_ar(0)
            v_unit(0)
            qk_unit(1, 0, xs["c", 1], True)
            softmax(0)
            qk_unit(1, 1, xs["p", 1], False)
            qk_unit(1, 2, xs["n", 1], False)
            stats_and_ar(1)
            v_unit(1)
            pass2(0)
            softmax(1)
            pass2(1)

    nc.compile()
    return nc


def _prep_inputs(inputs):
    """Build per-core in_maps from full inputs."""
    x_curr = np.asarray(inputs["x_curr"], np.float32)
    x_prev = np.asarray(inputs["x_prev"], np.float32)
    x_next = np.asarray(inputs["x_next"], np.float32)
    w_q = np.asarray(inputs["w_q"], np.float32)
    w_q_dw = np.asarray(inputs["w_q_dw"], np.float32)
    w_kv_prev = np.asarray(inputs["w_kv_prev"], np.float32)
    w_kv_dw_prev = np.asarray(inputs["w_kv_dw_prev"], np.float32)
    w_kv_next = np.asarray(inputs["w_kv_next"], np.float32)
    w_kv_dw_next = np.asarray(inputs["w_kv_dw_next"], np.float32)
    w_proj = np.asarray(inputs["w_proj"], np.float32)
    temperature = np.asarray(inputs["temperature"], np.float32)

    # tap order implied by PAIR_T, with a zero pad tap in slot 9
    tap_order = []
    for t0, t1, _ in PAIR_T:
        tap_order.append(t0)
        tap_order.append(t1)

    def w3block(w1, wdw):
        # [C(in), 10 taps, C(out)], fp8, scaled by WSCALE
        wdw9 = wdw.reshape(C, 9)
        blk = np.zeros((C, 10, C), np.float32)
        for j, t in enumerate(tap_order):
            if t is None:
                continue
            blk[:, j, :] = WSCALE * np.einsum("oc,o->co", w1, wdw9[:, t])
        return blk

    w3qk = np.stack([
        w3block(w_q, w_q_dw),
        w3block(w_kv_prev[0:C], w_kv_dw_prev[0:C]),
        w3block(w_kv_next[0:C], w_kv_dw_next[0:C]),
    ], axis=1).astype(ml_dtypes.float8_e4m3)
    def w3nat(w1, wdw):
        return np.einsum("oc,ot->cto", w1, wdw.reshape(C, 9))

    w3v = np.stack([
        w3nat(w_kv_prev[C:2 * C], w_kv_dw_prev[C:2 * C]),
        w3nat(w_kv_next[C:2 * C], w_kv_dw_next[C:2 * C]),
    ], axis=1).astype(ml_dtypes.bfloat16)

    wpt = np.ascontiguousarray(w_proj.T)
    tmpv = np.repeat(temperature.reshape(HEADS), CH).reshape(C, 1)
    tmpv = np.ascontiguousarray(tmpv, np.float32)
    hmk = np.zeros((C, HEADS), np.float32)
    for h in range(HEADS):
        hmk[h * CH:(h + 1) * CH, h] = 1.0
    bmk = np.zeros((C, C), np.float32)
    for h in range(HEADS):
        bmk[h * CH:(h + 1) * CH, h * CH:(h + 1) * CH] = 1.0

    def strip(x, c, dt_):
        # guard-padded flat strip, quantized to dt_
        r0 = c * RPC - 1
        r1 = c * RPC + RPC + 1
        rows = np.zeros((B, C, SROWS, PITCH), np.float32)
        lo, hi = max(r0, 0), min(r1, H)
        rows[:, :, lo - r0:lo - r0 + hi - lo, 0:W] = x[:, :, lo:hi, :]
        out = np.zeros((B, C, XLEN), np.float32)
        out[:, :, LEAD:LEAD + SROWS * PITCH] = rows.reshape(B, C, -1)
        return out.astype(dt_)

    in_maps = []
    for c in range(N_CORES):
        f8, b16 = ml_dtypes.float8_e4m3, ml_dtypes.bfloat16
        in_maps.append({
            "x8c": strip(x_curr, c, f8),
            "x8p": strip(x_prev, c, f8),
            "x8n": strip(x_next, c, f8),
            "xbp": strip(x_prev, c, b16),
            "xbn": strip(x_next, c, b16),
            "w3qk": w3qk,
            "w3v": w3v,
            "wpt": wpt.astype(np.float32),
            "tmp": tmpv,
            "idn": np.eye(C, dtype=np.float32),
            "hmk": hmk,
            "bmk": bmk,
        })
    return in_maps


def kernel(**inputs):
    if "nc" not in _CACHE:
        _CACHE["nc"] = build_kernel()
    nc = _CACHE["nc"]
    in_maps = _prep_inputs(inputs)
    res = run_bass_kernel_spmd(nc, in_maps, core_ids=list(range(N_CORES)))
    out = np.empty((B, C, H, W), np.float32)
    for c in range(N_CORES):
        out[:, :, c * RPC:(c + 1) * RPC, :] = res.results[c]["y"]
    return out


if __name__ == "__main__":
    rng = np.random.default_rng(0)
    inputs = {
        "x_curr": rng.standard_normal((B, C, H, W), np.float32),
        "x_prev": rng.standard_normal((B, C, H, W), np.float32),
        "x_next": rng.standard_normal((B, C, H, W), np.float32),
        "w_q": rng.standard_normal((C, C), np.float32) * 0.02,
        "w_q_dw": rng.standard_normal((C, 1, 3, 3), np.float32) * 0.02,
        "w_kv_prev": rng.standard_normal((2 * C, C), np.float32) * 0.02,
        "w_kv_dw_prev": rng.standard_normal((2 * C, 1, 3, 3), np.float32) * 0.02,
        "w_kv_next": rng.standard_normal((2 * C, C), np.float32) * 0.02,
        "w_kv_dw_next": rng.standard_normal((2 * C, 1, 3, 3), np.float32) * 0.02,
        "w_proj": rng.standard_normal((C, C), np.float32) * 0.02,
        "temperature": np.ones((HEADS, 1, 1), np.float32),
    }
    out = kernel(**inputs)
    print("out", out.shape, out.dtype, np.abs(out).max())


# revision 21
# speedup vs baseline: 4.8089x; 1.0730x over previous
"""Co-Attention kernel for Trainium2, 8-core SPMD.

Sharding: spatial (H rows) across 8 cores; 32 rows/core with 1-row halo.
Per-core pipeline (all fused, single launch):
  - conv1x1+dwconv3x3 folded into shifted matmuls with combined weights
    W3_t[o,c] = W1[o,c] * wdw[o,t]; operands quantized to fp8e4 and run in
    DoubleRow perf mode with TWO taps packed per matmul via the k-tile dim
    (rhs dim1 = tap-pair with a strided/overlapping access pattern)
  - q/k: PSUM -> bf16 -> PE transpose -> [n,c] tiles -> Gram matrices
    (q@kT + self-Grams for the L2 norms) accumulated on PE over the
    core's spatial shard; v_prev+v_next accumulate into one PSUM chain
    and land in a bf16 SBUF-resident strip
  - per-batch AllReduce of the tiny Gram/norm stats across the 8 cores,
    software-pipelined against the other batch's conv pass
  - on-chip double softmax (block-diagonal channel attention)
  - output = (w_proj @ blockdiag(attn_co)) @ v_sum, one matmul per 2 rows
"""

import os
import sys

sys.path.insert(0, "/opt/trn_rl_repo")

import numpy as np
import ml_dtypes

import concourse.bacc as bacc
import concourse.bass as bass
import concourse.tile as tile
from concourse import mybir
from concourse.ap import AP
from concourse.bass_utils import run_bass_kernel_spmd

# problem constants
B, C, H, W = 2, 96, 256, 256
HEADS = 4
CH = C // HEADS
N_CORES = 8
RPC = H // N_CORES          # rows per core (32)
SROWS = RPC + 2             # strip rows incl halo (34)
PITCH = W + 2               # guarded row pitch (258)
LEAD = 2                    # leading guard pad
XLEN = LEAD + SROWS * PITCH + 2  # strip flat length (8776)
NCH = RPC // 2              # 2-row chunks per core (16)
NTILES = RPC * 2            # 128-wide transpose tiles per unit per b (64)
VLEN = RPC * W              # v_sum flat length per b (8192)
WSCALE = 4096.0             # fp8 weight scale (exact power of 2)

F32 = mybir.dt.float32
BF16 = mybir.dt.bfloat16
F8 = mybir.dt.float8e4
DR = mybir.MatmulPerfMode.DoubleRow

# tap offsets (cross-correlation, matching jax.lax.conv_general_dilated)
TAPS = [(ky - 1) * PITCH + (kx - 1) for ky in range(3) for kx in range(3)]
# tap-pair packing for DoubleRow: (first tap, second tap, rhs dim1 stride);
# last pair duplicates tap 8 with zero weights (stride 0)
PAIR_T = [(0, 1, 1), (2, 5, PITCH), (3, 4, 1), (6, 7, 1), (8, None, 0)]

_CACHE = {}


def rowoff(r):
    return LEAD + r * PITCH


def pair_rhs(xt, j, m):
    """Overlapping strided rhs AP [C, ktile=2, rows=2, cols=256] for the
    DoubleRow tap-pair matmul of chunk j, pair m."""
    t0, _, dstride = PAIR_T[m]
    base = xt[:]
    off = base.offset + rowoff(1 + 2 * j) + TAPS[t0]
    ppair = list(base.ap[0])
    return AP(base.tensor, off,
              [ppair, [dstride, 2], [PITCH, 2], [1, W]])


def row2_rhs(xt, j, t):
    """Plain 2-row rhs AP [C, rows=2, cols=256] for tap t of chunk j."""
    base = xt[:]
    off = base.offset + rowoff(1 + 2 * j) + TAPS[t]
    ppair = list(base.ap[0])
    return AP(base.tensor, off, [ppair, [PITCH, 2], [1, W]])


def build_kernel():
    SKIP_AR = bool(os.environ.get("SKIP_AR"))
    nc = bacc.Bacc("TRN2", target_bir_lowering=False, debug=False,
                   num_devices=N_CORES)

    x8c = nc.declare_dram_parameter("x8c", [B, C, XLEN], F8, isOutput=False)
    x8p = nc.declare_dram_parameter("x8p", [B, C, XLEN], F8, isOutput=False)
    x8n = nc.declare_dram_parameter("x8n", [B, C, XLEN], F8, isOutput=False)
    w3qk = nc.declare_dram_parameter("w3qk", [C, 3, 10, C], F8, isOutput=False)
    xbp = nc.declare_dram_parameter("xbp", [B, C, XLEN], BF16, isOutput=False)
    xbn = nc.declare_dram_parameter("xbn", [B, C, XLEN], BF16, isOutput=False)
    w3v = nc.declare_dram_parameter("w3v", [C, 2, 9, C], BF16, isOutput=False)
    wpt = nc.declare_dram_parameter("wpt", [C, C], F32, isOutput=False)
    tmp = nc.declare_dram_parameter("tmp", [C, 1], F32, isOutput=False)
    idn = nc.declare_dram_parameter("idn", [C, C], F32, isOutput=False)
    hmk = nc.declare_dram_parameter("hmk", [C, HEADS], F32, isOutput=False)
    bmk = nc.declare_dram_parameter("bmk", [C, C], F32, isOutput=False)
    y = nc.declare_dram_parameter("y", [B, C, RPC, W], F32, isOutput=True)

    ar_in = [nc.dram_tensor(f"ar_in{b}", [C, 195], F32) for b in range(B)]
    ar_out = [nc.dram_tensor(f"ar_out{b}", [C, 195], F32, addr_space="Shared")
              for b in range(B)]

    with tile.TileContext(nc) as tc:
        with (
            tc.tile_pool(name="singles", bufs=1) as singles,
            tc.tile_pool(name="xpool", bufs=1) as xpool,
            tc.tile_pool(name="dwsb", bufs=3) as dwsbp,
            tc.tile_pool(name="qstorep", bufs=2) as qstorep,
            tc.tile_pool(name="kstore", bufs=3) as kstorep,
            tc.tile_pool(name="small", bufs=2) as smallp,
            tc.tile_pool(name="outp", bufs=2) as outp,
            tc.tile_pool(name="psdw", bufs=3, space="PSUM") as psdw,
            tc.tile_pool(name="pstp", bufs=2, space="PSUM") as pstp,
            tc.tile_pool(name="psg", bufs=1, space="PSUM") as psg,
        ):
            # ---- constants ----
            w3qk_sb = singles.tile([C, 3, 10, C], F8)
            nc.sync.dma_start(out=w3qk_sb[:], in_=w3qk[:, :, :, :])
            w3v_sb = singles.tile([C, 2, 9, C], BF16)
            nc.sync.dma_start(out=w3v_sb[:], in_=w3v[:, :, :, :])
            wpt_sb = singles.tile([C, C], F32)
            nc.sync.dma_start(out=wpt_sb[:], in_=wpt[:, :])
            temp_sb = singles.tile([C, 1], F32)
            nc.sync.dma_start(out=temp_sb[:], in_=tmp[:, :])
            ident = singles.tile([C, C], F32)
            nc.sync.dma_start(out=ident[:], in_=idn[:, :])
            identb = singles.tile([C, C], BF16)
            nc.vector.tensor_copy(out=identb[:], in_=ident[:])
            hmask = singles.tile([C, HEADS], F32)
            nc.sync.dma_start(out=hmask[:], in_=hmk[:, :])
            bmask = singles.tile([C, C], F32)
            nc.sync.dma_start(out=bmask[:], in_=bmk[:, :])

            # input strips: all 6 up-front (no deps, fills SBUF early)
            xs = {}
            for b in range(B):
                for key, src in (("c", x8c), ("p", x8p), ("n", x8n)):
                    pass
                for key, src in (("c", x8c), ("p", x8p), ("n", x8n),
                                 ("bp", xbp), ("bn", xbn)):
                    dt_ = BF16 if key.startswith("b") else F8
                    t = xpool.tile([C, XLEN], dt_, tag=f"x{key}{b}",
                                   name="xstrip")
                    nc.sync.dma_start(out=t[:], in_=src[b])
                    xs[key, b] = t

            # persistent accumulators
            v_sum = singles.tile([C, B, VLEN], BF16)
            ar_sb = singles.tile([C, B, 195], F32)
            gram_sb = singles.tile([C, B, 5, C], F32)
            arr_sb = singles.tile([C, B, 195], F32)
            mct_sb = singles.tile([C, B, C], BF16)

            qstore = {}

            # ------------ pass 1 pieces ------------
            def qk_unit(b, u, xt, is_q):
                """q or k unit: 5-stage chunk pipeline.
                stages: taps(PE) -> evac(Act) -> transpose(PE) ->
                        ucopy(DVE) -> grams(PE)."""
                if is_q:
                    qstore[b] = qstorep.tile([128, NTILES, C], BF16,
                                             tag="qstore", name="qstore")
                    g_self = psg.tile([C, C], F32, tag="g")
                    g_cross = None
                else:
                    g_self = psg.tile([C, C], F32, tag="g")
                    g_cross = psg.tile([C, C], F32, tag="g2")
                dw = [None] * NCH
                sb = [None] * NCH
                tpt = [None] * NCH
                kt = [None] * NCH

                def taps(j):
                    dwps = psdw.tile([C, 2, W], F32, tag="dwps")
                    for m in range(5):
                        nc.tensor.matmul(
                            dwps[:], lhsT=w3qk_sb[:, u, 2 * m:2 * m + 2, :],
                            rhs=pair_rhs(xt, j, m),
                            start=(m == 0), stop=(m == 4), perf_mode=DR)
                    dw[j] = dwps

                def evac(j):
                    dwsb = dwsbp.tile([C, 2, W], BF16)
                    nc.scalar.copy(out=dwsb[:], in_=dw[j][:])
                    sb[j] = dwsb

                def transp(j):
                    tp = pstp.tile([128, 4, C], BF16, tag="tp")
                    for r2 in range(2):
                        for hf in range(2):
                            nc.tensor.transpose(
                                tp[:, 2 * r2 + hf, :],
                                sb[j][:, r2, 128 * hf:128 * hf + 128],
                                identb[:])
                    tpt[j] = tp

                def ucopy(j):
                    if is_q:
                        dst = qstore[b][:, 4 * j:4 * j + 4, :]
                    else:
                        kt[j] = kstorep.tile([128, 4, C], BF16, tag="kT",
                                             name="kt")
                        dst = kt[j][:]
                    nc.vector.tensor_copy(out=dst, in_=tpt[j][:])

                def gram(j):
                    for i in range(4):
                        gi = 4 * j + i
                        st = (gi == 0)
                        sp = (gi == NTILES - 1)
                        if is_q:
                            nc.tensor.matmul(
                                g_self[:], lhsT=qstore[b][:, gi, :],
                                rhs=qstore[b][:, gi, :], start=st, stop=sp,
                                skip_group_check=True)
                        else:
                            nc.tensor.matmul(
                                g_cross[:], lhsT=qstore[b][:, gi, :],
                                rhs=kt[j][:, i, :], start=st, stop=sp,
                                skip_group_check=True)
                            nc.tensor.matmul(
                                g_self[:], lhsT=kt[j][:, i, :],
                                rhs=kt[j][:, i, :], start=st, stop=sp,
                                skip_group_check=True)

                stages = [taps, evac, transp, ucopy, gram]
                for j in range(NCH + 4):
                    for s, fn in enumerate(stages):
                        if 0 <= j - s < NCH:
                            fn(j - s)

                # evacuate gram psums
                if is_q:
                    nc.vector.tensor_copy(out=gram_sb[:, b, 0, :],
                                          in_=g_self[:])
                else:
                    slot = 1 if u == 1 else 3
                    nc.vector.tensor_copy(out=gram_sb[:, b, slot, :],
                                          in_=g_cross[:])
                    nc.vector.tensor_copy(out=gram_sb[:, b, slot + 1, :],
                                          in_=g_self[:])

            def v_unit(b, extras=None):
                """fused v_prev+v_next: 10 DR tap matmuls into one PSUM
                accumulation per chunk, Pool evacuates to bf16 v_sum."""
                dw = [None] * NCH

                def taps(j):
                    dwps = psdw.tile([C, 2, W], F32, tag="dwps")
                    for part, key in enumerate(("bp", "bn")):
                        for t in range(9):
                            nc.tensor.matmul(
                                dwps[:],
                                lhsT=w3v_sb[:, part, t, :],
                                rhs=row2_rhs(xs[key, b], j, t),
                                start=(part == 0 and t == 0),
                                stop=(part == 1 and t == 8))
                    dw[j] = dwps

                def evac(j):
                    nc.vector.tensor_copy(
                        out=v_sum[:, b, 2 * W * j:2 * W * (j + 1)].rearrange(
                            "p (r w) -> p r w", w=W),
                        in_=dw[j][:])

                for j in range(NCH + 1):
                    if j < NCH:
                        taps(j)
                    if j >= 1:
                        evac(j - 1)
                    if extras and j < len(extras):
                        extras[j]()

            def stats_and_ar(b):
                # diag extraction via masked reduce + AllReduce kickoff
                scr = smallp.tile([C, C], F32, tag="scr")
                for k, slot in enumerate((0, 2, 4)):
                    nc.vector.tensor_mul(out=scr[:],
                                         in0=gram_sb[:, b, slot, :],
                                         in1=ident[:])
                    nc.vector.reduce_sum(out=ar_sb[:, b, 192 + k:193 + k],
                                         in_=scr[:],
                                         axis=mybir.AxisListType.X)
                nc.vector.tensor_copy(out=ar_sb[:, b, 0:96],
                                      in_=gram_sb[:, b, 1, :])
                nc.vector.tensor_copy(out=ar_sb[:, b, 96:192],
                                      in_=gram_sb[:, b, 3, :])
                if SKIP_AR:
                    nc.vector.tensor_copy(out=arr_sb[:, b, :],
                                          in_=ar_sb[:, b, :])
                else:
                    nc.sync.dma_start(out=ar_in[b][:, :], in_=ar_sb[:, b, :])
                    nc.gpsimd.collective_compute(
                        "AllReduce", mybir.AluOpType.add,
                        replica_groups=[list(range(N_CORES))],
                        ins=[ar_in[b][:, :]], outs=[ar_out[b][:, :]],
                    )
                    nc.sync.dma_start(out=arr_sb[:, b, :],
                                      in_=ar_out[b][:, :])

            def softmax(b):
                rinv = smallp.tile([C, 3], F32, tag="rinv")
                nc.scalar.activation(out=rinv[:], in_=arr_sb[:, b, 192:195],
                                     func=mybir.ActivationFunctionType.Sqrt)
                nc.vector.tensor_scalar_max(out=rinv[:], in0=rinv[:],
                                            scalar1=1e-12)
                nc.vector.reciprocal(out=rinv[:], in_=rinv[:])
                rqt = smallp.tile([C, 1], F32, tag="rqt")
                nc.vector.tensor_mul(out=rqt[:], in0=rinv[:, 0:1],
                                     in1=temp_sb[:])

                ee = smallp.tile([C, 2, C], F32, tag="ee")
                ssum = smallp.tile([C, 2, HEADS], F32, tag="ssum")
                for s in range(2):
                    logits = smallp.tile([C, C], F32, tag="logits")
                    nc.vector.tensor_scalar_mul(
                        out=logits[:], in0=arr_sb[:, b, 96 * s:96 * s + 96],
                        scalar1=rqt[:])
                    # column scale via transpose sandwich
                    lt_ps = psg.tile([C, C], F32, tag="g")
                    nc.tensor.transpose(lt_ps[:], logits[:], ident[:])
                    lts = smallp.tile([C, C], F32, tag="lts")
                    nc.vector.tensor_scalar_mul(out=lts[:], in0=lt_ps[:],
                                                scalar1=rinv[:, 1 + s:2 + s])
                    lt2_ps = psg.tile([C, C], F32, tag="g2")
                    nc.tensor.transpose(lt2_ps[:], lts[:], ident[:])
                    nc.vector.tensor_copy(out=logits[:], in_=lt2_ps[:])
                    nc.scalar.activation(out=ee[:, s, :], in_=logits[:],
                                         func=mybir.ActivationFunctionType.Exp)
                    nc.vector.reduce_sum(
                        out=ssum[:, s, :],
                        in_=ee[:, s, :].rearrange("p (h d) -> p h d", h=HEADS),
                        axis=mybir.AxisListType.X)
                # rpn = 1/(Sp*Sn) per block
                rpn = smallp.tile([C, HEADS], F32, tag="rpn")
                nc.vector.tensor_mul(out=rpn[:], in0=ssum[:, 0, :],
                                     in1=ssum[:, 1, :])
                nc.vector.reciprocal(out=rpn[:], in_=rpn[:])
                scrh = smallp.tile([C, HEADS], F32, tag="scrh")
                rc1 = smallp.tile([C, 1], F32, tag="rc1")
                nc.vector.tensor_mul(out=scrh[:], in0=rpn[:], in1=hmask[:])
                nc.vector.reduce_sum(out=rc1[:], in_=scrh[:],
                                     axis=mybir.AxisListType.X)
                pp = smallp.tile([C, C], F32, tag="pp")
                nc.vector.tensor_mul(out=pp[:], in0=ee[:, 0, :],
                                     in1=ee[:, 1, :])
                nc.vector.tensor_scalar_mul(out=pp[:], in0=pp[:],
                                            scalar1=rc1[:])
                e2 = smallp.tile([C, C], F32, tag="e2")
                nc.scalar.activation(out=e2[:], in_=pp[:],
                                     func=mybir.ActivationFunctionType.Exp)
                s2 = smallp.tile([C, HEADS], F32, tag="s2")
                nc.vector.reduce_sum(
                    out=s2[:],
                    in_=e2[:].rearrange("p (h d) -> p h d", h=HEADS),
                    axis=mybir.AxisListType.X)
                nc.vector.reciprocal(out=s2[:], in_=s2[:])
                rc2 = smallp.tile([C, 1], F32, tag="rc2")
                nc.vector.tensor_mul(out=scrh[:], in0=s2[:], in1=hmask[:])
                nc.vector.reduce_sum(out=rc2[:], in_=scrh[:],
                                     axis=mybir.AxisListType.X)
                bd = smallp.tile([C, C], F32, tag="bd")
                nc.vector.tensor_scalar_mul(out=bd[:], in0=e2[:],
                                            scalar1=rc2[:])
                nc.vector.tensor_mul(out=bd[:], in0=bd[:], in1=bmask[:])
                mct_ps = psg.tile([C, C], F32, tag="g2")
                nc.tensor.matmul(mct_ps[:], lhsT=bd[:], rhs=wpt_sb[:],
                                 start=True, stop=True)
                nc.vector.tensor_copy(out=mct_sb[:, b, :], in_=mct_ps[:])

            def pass2_chunks(b):
                ops = [None] * NCH

                def mm(j):
                    t = psdw.tile([C, 2, W], F32, tag="dwps")
                    nc.tensor.matmul(
                        t[:], lhsT=mct_sb[:, b, :],
                        rhs=v_sum[:, b, 2 * W * j:2 * W * (j + 1)],
                        start=True, stop=True)
                    ops[j] = t

                def evac(j):
                    osb = outp.tile([C, 2, W], F32)
                    if j % 2 == 0:
                        nc.scalar.copy(out=osb[:], in_=ops[j][:])
                    else:
                        nc.vector.tensor_copy(out=osb[:], in_=ops[j][:])
                    nc.sync.dma_start(out=y[b, :, 2 * j:2 * j + 2, :],
                                      in_=osb[:])

                def step(j):
                    def go():
                        if j < NCH:
                            mm(j)
                        if j >= 1:
                            evac(j - 1)
                    return go
                return [step(j) for j in range(NCH + 1)]

            def pass2(b):
                for fn in pass2_chunks(b):
                    fn()

            # ------------ emission schedule ------------
            # b0 conv -> AR(b0) overlaps [v(b0), q(b1)] -> softmax(b0)
            # -> pass2(b0) after kn(b1) -> AR(b1) overlaps [v(b1), pass2(b0)]
            qk_unit(0, 0, xs["c", 0], True)
            qk_unit(0, 1, xs["p", 0], False)
            qk_unit(0, 2, xs["n", 0], False)
            stats_and_ar(0)
            v_unit(0)
            qk_unit(1, 0, xs["c", 1], True)
            softmax(0)
            qk_unit(1, 1, xs["p", 1], False)
            qk_unit(1, 2, xs["n", 1], False)
            stats_and_ar(1)
            v_unit(1, extras=pass2_chunks(0))
            softmax(1)
            pass2(1)

    nc.compile()
    return nc


def _prep_inputs(inputs):
    """Build per-core in_maps from full inputs."""
    x_curr = np.asarray(inputs["x_curr"], np.float32)
    x_prev = np.asarray(inputs["x_prev"], np.float32)
    x_next = np.asarray(inputs["x_next"], np.float32)
    w_q = np.asarray(inputs["w_q"], np.float32)
    w_q_dw = np.asarray(inputs["w_q_dw"], np.float32)
    w_kv_prev = np.asarray(inputs["w_kv_prev"], np.float32)
    w_kv_dw_prev = np.asarray(inputs["w_kv_dw_prev"], np.float32)
    w_kv_next = np.asarray(inputs["w_kv_next"], np.float32)
    w_kv_dw_next = np.asarray(inputs["w_kv_dw_next"], np.float32)
    w_proj = np.asarray(inputs["w_proj"], np.float32)
    temperature = np.asarray(inputs["temperature"], np.float32)

    # tap order implied by PAIR_T, with a zero pad tap in slot 9
    tap_order = []
    for t0, t1, _ in PAIR_T:
        tap_order.append(t0)
        tap_order.append(t1)

    def w3block(w1, wdw):
        # [C(in), 10 taps, C(out)], fp8, scaled by WSCALE
        wdw9 = wdw.reshape(C, 9)
        blk = np.zeros((C, 10, C), np.float32)
        for j, t in enumerate(tap_order):
            if t is None:
                continue
            blk[:, j, :] = WSCALE * np.einsum("oc,o->co", w1, wdw9[:, t])
        return blk

    w3qk = np.stack([
        w3block(w_q, w_q_dw),
        w3block(w_kv_prev[0:C], w_kv_dw_prev[0:C]),
        w3block(w_kv_next[0:C], w_kv_dw_next[0:C]),
    ], axis=1).astype(ml_dtypes.float8_e4m3)
    def w3nat(w1, wdw):
        return np.einsum("oc,ot->cto", w1, wdw.reshape(C, 9))

    w3v = np.stack([
        w3nat(w_kv_prev[C:2 * C], w_kv_dw_prev[C:2 * C]),
        w3nat(w_kv_next[C:2 * C], w_kv_dw_next[C:2 * C]),
    ], axis=1).astype(ml_dtypes.bfloat16)

    wpt = np.ascontiguousarray(w_proj.T)
    tmpv = np.repeat(temperature.reshape(HEADS), CH).reshape(C, 1)
    tmpv = np.ascontiguousarray(tmpv, np.float32)
    hmk = np.zeros((C, HEADS), np.float32)
    for h in range(HEADS):
        hmk[h * CH:(h + 1) * CH, h] = 1.0
    bmk = np.zeros((C, C), np.float32)
    for h in range(HEADS):
        bmk[h * CH:(h + 1) * CH, h * CH:(h + 1) * CH] = 1.0

    def strip(x, c, dt_):
        # guard-padded flat strip, quantized to dt_
        r0 = c * RPC - 1
        r1 = c * RPC + RPC + 1
        rows = np.zeros((B, C, SROWS, PITCH), np.float32)
        lo, hi = max(r0, 0), min(r1, H)
        rows[:, :, lo - r0:lo - r0 + hi - lo, 0:W] = x[:, :, lo:hi, :]
        out = np.zeros((B, C, XLEN), np.float32)
        out[:, :, LEAD:LEAD + SROWS * PITCH] = rows.reshape(B, C, -1)
        return out.astype(dt_)

    in_maps = []
    for c in range(N_CORES):
        f8, b16 = ml_dtypes.float8_e4m3, ml_dtypes.bfloat16
        in_maps.append({
            "x8c": strip(x_curr, c, f8),
            "x8p": strip(x_prev, c, f8),
            "x8n": strip(x_next, c, f8),
            "xbp": strip(x_prev, c, b16),
            "xbn": strip(x_next, c, b16),
            "w3qk": w3qk,
            "w3v": w3v,
            "wpt": wpt.astype(np.float32),
            "tmp": tmpv,
            "idn": np.eye(C, dtype=np.float32),
            "hmk": hmk,
            "bmk": bmk,
        })
    return in_maps


def kernel(**inputs):
    if "nc" not in _CACHE:
        _CACHE["nc"] = build_kernel()
    nc = _CACHE["nc"]
    in_maps = _prep_inputs(inputs)
    res = run_bass_kernel_spmd(nc, in_maps, core_ids=list(range(N_CORES)))
    out = np.empty((B, C, H, W), np.float32)
    for c in range(N_CORES):
        out[:, :, c * RPC:(c + 1) * RPC, :] = res.results[c]["y"]
    return out


if __name__ == "__main__":
    rng = np.random.default_rng(0)
    inputs = {
        "x_curr": rng.standard_normal((B, C, H, W), np.float32),
        "x_prev": rng.standard_normal((B, C, H, W), np.float32),
        "x_next": rng.standard_normal((B, C, H, W), np.float32),
        "w_q": rng.standard_normal((C, C), np.float32) * 0.02,
        "w_q_dw": rng.standard_normal((C, 1, 3, 3), np.float32) * 0.02,
        "w_kv_prev": rng.standard_normal((2 * C, C), np.float32) * 0.02,
        "w_kv_dw_prev": rng.standard_normal((2 * C, 1, 3, 3), np.float32) * 0.02,
        "w_kv_next": rng.standard_normal((2 * C, C), np.float32) * 0.02,
        "w_kv_dw_next": rng.standard_normal((2 * C, 1, 3, 3), np.float32) * 0.02,
        "w_proj": rng.standard_normal((C, C), np.float32) * 0.02,
        "temperature": np.ones((HEADS, 1, 1), np.float32),
    }
    out = kernel(**inputs)
    print("out", out.shape, out.dtype, np.abs(out).max())


# revision 22
# speedup vs baseline: 4.8892x; 1.0167x over previous
"""Co-Attention kernel for Trainium2, 8-core SPMD.

Sharding: spatial (H rows) across 8 cores; 32 rows/core with 1-row halo.
Per-core pipeline (all fused, single launch):
  - conv1x1+dwconv3x3 folded into shifted matmuls with combined weights
    W3_t[o,c] = W1[o,c] * wdw[o,t]; operands quantized to fp8e4 and run in
    DoubleRow perf mode with TWO taps packed per matmul via the k-tile dim
    (rhs dim1 = tap-pair with a strided/overlapping access pattern)
  - q/k: PSUM -> bf16 -> PE transpose -> [n,c] tiles -> Gram matrices
    (q@kT + self-Grams for the L2 norms) accumulated on PE over the
    core's spatial shard; v_prev+v_next accumulate into one PSUM chain
    and land in a bf16 SBUF-resident strip
  - per-batch AllReduce of the tiny Gram/norm stats across the 8 cores,
    software-pipelined against the other batch's conv pass
  - on-chip double softmax (block-diagonal channel attention)
  - output = (w_proj @ blockdiag(attn_co)) @ v_sum, one matmul per 2 rows
"""

import os
import sys

sys.path.insert(0, "/opt/trn_rl_repo")

import numpy as np
import ml_dtypes

import concourse.bacc as bacc
import concourse.bass as bass
import concourse.tile as tile
from concourse import mybir
from concourse.ap import AP
from concourse.bass_utils import run_bass_kernel_spmd

# problem constants
B, C, H, W = 2, 96, 256, 256
HEADS = 4
CH = C // HEADS
N_CORES = 8
RPC = H // N_CORES          # rows per core (32)
SROWS = RPC + 2             # strip rows incl halo (34)
PITCH = W + 2               # guarded row pitch (258)
LEAD = 2                    # leading guard pad
XLEN = LEAD + SROWS * PITCH + 2  # strip flat length (8776)
NCH = RPC // 2              # 2-row chunks per core (16)
NTILES = RPC * 2            # 128-wide transpose tiles per unit per b (64)
VLEN = RPC * W              # v_sum flat length per b (8192)
WSCALE = 4096.0             # fp8 weight scale (exact power of 2)

F32 = mybir.dt.float32
BF16 = mybir.dt.bfloat16
F8 = mybir.dt.float8e4
DR = mybir.MatmulPerfMode.DoubleRow

# tap offsets (cross-correlation, matching jax.lax.conv_general_dilated)
TAPS = [(ky - 1) * PITCH + (kx - 1) for ky in range(3) for kx in range(3)]
# tap-pair packing for DoubleRow: (first tap, second tap, rhs dim1 stride);
# last pair duplicates tap 8 with zero weights (stride 0)
PAIR_T = [(0, 1, 1), (2, 5, PITCH), (3, 4, 1), (6, 7, 1), (8, None, 0)]

_CACHE = {}


def rowoff(r):
    return LEAD + r * PITCH


def pair_rhs(xt, j, m):
    """Overlapping strided rhs AP [C, ktile=2, rows=2, cols=256] for the
    DoubleRow tap-pair matmul of chunk j, pair m."""
    t0, _, dstride = PAIR_T[m]
    base = xt[:]
    off = base.offset + rowoff(1 + 2 * j) + TAPS[t0]
    ppair = list(base.ap[0])
    return AP(base.tensor, off,
              [ppair, [dstride, 2], [PITCH, 2], [1, W]])


def row2_rhs(xt, j, t):
    """Plain 2-row rhs AP [C, rows=2, cols=256] for tap t of chunk j."""
    base = xt[:]
    off = base.offset + rowoff(1 + 2 * j) + TAPS[t]
    ppair = list(base.ap[0])
    return AP(base.tensor, off, [ppair, [PITCH, 2], [1, W]])


def build_kernel():
    SKIP_AR = bool(os.environ.get("SKIP_AR"))
    nc = bacc.Bacc("TRN2", target_bir_lowering=False, debug=False,
                   num_devices=N_CORES)

    x8c = nc.declare_dram_parameter("x8c", [B, C, XLEN], F8, isOutput=False)
    x8p = nc.declare_dram_parameter("x8p", [B, C, XLEN], F8, isOutput=False)
    x8n = nc.declare_dram_parameter("x8n", [B, C, XLEN], F8, isOutput=False)
    w3qk = nc.declare_dram_parameter("w3qk", [C, 3, 10, C], F8, isOutput=False)
    xbp = nc.declare_dram_parameter("xbp", [B, C, XLEN], BF16, isOutput=False)
    xbn = nc.declare_dram_parameter("xbn", [B, C, XLEN], BF16, isOutput=False)
    w3v = nc.declare_dram_parameter("w3v", [C, 2, 9, C], BF16, isOutput=False)
    wpt = nc.declare_dram_parameter("wpt", [C, C], F32, isOutput=False)
    tmp = nc.declare_dram_parameter("tmp", [C, 1], F32, isOutput=False)
    idn = nc.declare_dram_parameter("idn", [C, C], F32, isOutput=False)
    hmk = nc.declare_dram_parameter("hmk", [C, HEADS], F32, isOutput=False)
    bmk = nc.declare_dram_parameter("bmk", [C, C], F32, isOutput=False)
    y = nc.declare_dram_parameter("y", [B, C, RPC, W], F32, isOutput=True)

    ar_in = [nc.dram_tensor(f"ar_in{b}", [C, 195], F32) for b in range(B)]
    ar_out = [nc.dram_tensor(f"ar_out{b}", [C, 195], F32, addr_space="Shared")
              for b in range(B)]

    with tile.TileContext(nc) as tc:
        with (
            tc.tile_pool(name="singles", bufs=1) as singles,
            tc.tile_pool(name="xpool", bufs=1) as xpool,
            tc.tile_pool(name="dwsb", bufs=3) as dwsbp,
            tc.tile_pool(name="qstorep", bufs=2) as qstorep,
            tc.tile_pool(name="kstore", bufs=3) as kstorep,
            tc.tile_pool(name="small", bufs=1) as smallp,
            tc.tile_pool(name="outp", bufs=3) as outp,
            tc.tile_pool(name="psdw", bufs=3, space="PSUM") as psdw,
            tc.tile_pool(name="pstp", bufs=2, space="PSUM") as pstp,
            tc.tile_pool(name="psg", bufs=1, space="PSUM") as psg,
        ):
            # ---- constants ----
            w3qk_sb = singles.tile([C, 3, 10, C], F8)
            nc.sync.dma_start(out=w3qk_sb[:], in_=w3qk[:, :, :, :])
            w3v_sb = singles.tile([C, 2, 9, C], BF16)
            nc.sync.dma_start(out=w3v_sb[:], in_=w3v[:, :, :, :])
            wpt_sb = singles.tile([C, C], F32)
            nc.sync.dma_start(out=wpt_sb[:], in_=wpt[:, :])
            temp_sb = singles.tile([C, 1], F32)
            nc.sync.dma_start(out=temp_sb[:], in_=tmp[:, :])
            ident = singles.tile([C, C], F32)
            nc.sync.dma_start(out=ident[:], in_=idn[:, :])
            identb = singles.tile([C, C], BF16)
            nc.vector.tensor_copy(out=identb[:], in_=ident[:])
            hmask = singles.tile([C, HEADS], F32)
            nc.sync.dma_start(out=hmask[:], in_=hmk[:, :])
            bmask = singles.tile([C, C], F32)
            nc.sync.dma_start(out=bmask[:], in_=bmk[:, :])

            # input strips: all 6 up-front (no deps, fills SBUF early)
            xs = {}
            for b in range(B):
                for key, src in (("c", x8c), ("p", x8p), ("n", x8n)):
                    pass
                for key, src in (("c", x8c), ("p", x8p), ("n", x8n),
                                 ("bp", xbp), ("bn", xbn)):
                    dt_ = BF16 if key.startswith("b") else F8
                    t = xpool.tile([C, XLEN], dt_, tag=f"x{key}{b}",
                                   name="xstrip")
                    nc.sync.dma_start(out=t[:], in_=src[b])
                    xs[key, b] = t

            # persistent accumulators
            v_sum = singles.tile([C, B, VLEN], BF16)
            ar_sb = singles.tile([C, B, 195], F32)
            gram_sb = singles.tile([C, B, 5, C], F32)
            arr_sb = singles.tile([C, B, 195], F32)
            mct_sb = singles.tile([C, B, C], BF16)

            qstore = {}

            # ------------ pass 1 pieces ------------
            def qk_unit(b, u, xt, is_q):
                """q or k unit: 5-stage chunk pipeline.
                stages: taps(PE) -> evac(Act) -> transpose(PE) ->
                        ucopy(DVE) -> grams(PE)."""
                if is_q:
                    qstore[b] = qstorep.tile([128, NTILES, C], BF16,
                                             tag="qstore", name="qstore")
                    g_self = psg.tile([C, C], F32, tag="g")
                    g_cross = None
                else:
                    g_self = psg.tile([C, C], F32, tag="g")
                    g_cross = psg.tile([C, C], F32, tag="g2")
                dw = [None] * NCH
                sb = [None] * NCH
                tpt = [None] * NCH
                kt = [None] * NCH

                def taps(j):
                    dwps = psdw.tile([C, 2, W], F32, tag="dwps")
                    for m in range(5):
                        nc.tensor.matmul(
                            dwps[:], lhsT=w3qk_sb[:, u, 2 * m:2 * m + 2, :],
                            rhs=pair_rhs(xt, j, m),
                            start=(m == 0), stop=(m == 4), perf_mode=DR)
                    dw[j] = dwps

                def evac(j):
                    dwsb = dwsbp.tile([C, 2, W], BF16)
                    nc.scalar.copy(out=dwsb[:], in_=dw[j][:])
                    sb[j] = dwsb

                def transp(j):
                    tp = pstp.tile([128, 4, C], BF16, tag="tp")
                    for r2 in range(2):
                        for hf in range(2):
                            nc.tensor.transpose(
                                tp[:, 2 * r2 + hf, :],
                                sb[j][:, r2, 128 * hf:128 * hf + 128],
                                identb[:])
                    tpt[j] = tp

                def ucopy(j):
                    if is_q:
                        dst = qstore[b][:, 4 * j:4 * j + 4, :]
                    else:
                        kt[j] = kstorep.tile([128, 4, C], BF16, tag="kT",
                                             name="kt")
                        dst = kt[j][:]
                    nc.vector.tensor_copy(out=dst, in_=tpt[j][:])

                def gram(j):
                    for i in range(4):
                        gi = 4 * j + i
                        st = (gi == 0)
                        sp = (gi == NTILES - 1)
                        if is_q:
                            nc.tensor.matmul(
                                g_self[:], lhsT=qstore[b][:, gi, :],
                                rhs=qstore[b][:, gi, :], start=st, stop=sp,
                                skip_group_check=True)
                        else:
                            nc.tensor.matmul(
                                g_cross[:], lhsT=qstore[b][:, gi, :],
                                rhs=kt[j][:, i, :], start=st, stop=sp,
                                skip_group_check=True)
                            nc.tensor.matmul(
                                g_self[:], lhsT=kt[j][:, i, :],
                                rhs=kt[j][:, i, :], start=st, stop=sp,
                                skip_group_check=True)

                stages = [taps, evac, transp, ucopy, gram]
                for j in range(NCH + 4):
                    for s, fn in enumerate(stages):
                        if 0 <= j - s < NCH:
                            fn(j - s)

                # evacuate gram psums
                if is_q:
                    nc.vector.tensor_copy(out=gram_sb[:, b, 0, :],
                                          in_=g_self[:])
                else:
                    slot = 1 if u == 1 else 3
                    nc.vector.tensor_copy(out=gram_sb[:, b, slot, :],
                                          in_=g_cross[:])
                    nc.vector.tensor_copy(out=gram_sb[:, b, slot + 1, :],
                                          in_=g_self[:])

            def v_unit(b, extras=None):
                """fused v_prev+v_next: 10 DR tap matmuls into one PSUM
                accumulation per chunk, Pool evacuates to bf16 v_sum."""
                dw = [None] * NCH

                def taps(j):
                    dwps = psdw.tile([C, 2, W], F32, tag="dwps")
                    for part, key in enumerate(("bp", "bn")):
                        for t in range(9):
                            nc.tensor.matmul(
                                dwps[:],
                                lhsT=w3v_sb[:, part, t, :],
                                rhs=row2_rhs(xs[key, b], j, t),
                                start=(part == 0 and t == 0),
                                stop=(part == 1 and t == 8))
                    dw[j] = dwps

                def evac(j):
                    nc.vector.tensor_copy(
                        out=v_sum[:, b, 2 * W * j:2 * W * (j + 1)].rearrange(
                            "p (r w) -> p r w", w=W),
                        in_=dw[j][:])

                for j in range(NCH + 1):
                    if j < NCH:
                        taps(j)
                    if j >= 1:
                        evac(j - 1)
                    if extras and j < len(extras):
                        extras[j]()

            def stats_and_ar(b):
                # diag extraction via masked reduce + AllReduce kickoff
                scr = smallp.tile([C, C], F32, tag="scr")
                for k, slot in enumerate((0, 2, 4)):
                    nc.vector.tensor_mul(out=scr[:],
                                         in0=gram_sb[:, b, slot, :],
                                         in1=ident[:])
                    nc.vector.reduce_sum(out=ar_sb[:, b, 192 + k:193 + k],
                                         in_=scr[:],
                                         axis=mybir.AxisListType.X)
                nc.vector.tensor_copy(out=ar_sb[:, b, 0:96],
                                      in_=gram_sb[:, b, 1, :])
                nc.vector.tensor_copy(out=ar_sb[:, b, 96:192],
                                      in_=gram_sb[:, b, 3, :])
                if SKIP_AR:
                    nc.vector.tensor_copy(out=arr_sb[:, b, :],
                                          in_=ar_sb[:, b, :])
                else:
                    nc.sync.dma_start(out=ar_in[b][:, :], in_=ar_sb[:, b, :])
                    nc.gpsimd.collective_compute(
                        "AllReduce", mybir.AluOpType.add,
                        replica_groups=[list(range(N_CORES))],
                        ins=[ar_in[b][:, :]], outs=[ar_out[b][:, :]],
                    )
                    nc.sync.dma_start(out=arr_sb[:, b, :],
                                      in_=ar_out[b][:, :])

            def softmax(b):
                rinv = smallp.tile([C, 3], F32, tag="rinv")
                nc.scalar.activation(out=rinv[:], in_=arr_sb[:, b, 192:195],
                                     func=mybir.ActivationFunctionType.Sqrt)
                nc.vector.tensor_scalar_max(out=rinv[:], in0=rinv[:],
                                            scalar1=1e-12)
                nc.vector.reciprocal(out=rinv[:], in_=rinv[:])
                rqt = smallp.tile([C, 1], F32, tag="rqt")
                nc.vector.tensor_mul(out=rqt[:], in0=rinv[:, 0:1],
                                     in1=temp_sb[:])

                ee = smallp.tile([C, 2, C], F32, tag="ee")
                ssum = smallp.tile([C, 2, HEADS], F32, tag="ssum")
                for s in range(2):
                    logits = smallp.tile([C, C], F32, tag="logits")
                    nc.vector.tensor_scalar_mul(
                        out=logits[:], in0=arr_sb[:, b, 96 * s:96 * s + 96],
                        scalar1=rqt[:])
                    # column scale via transpose sandwich
                    lt_ps = psg.tile([C, C], F32, tag="g")
                    nc.tensor.transpose(lt_ps[:], logits[:], ident[:])
                    lts = smallp.tile([C, C], F32, tag="lts")
                    nc.vector.tensor_scalar_mul(out=lts[:], in0=lt_ps[:],
                                                scalar1=rinv[:, 1 + s:2 + s])
                    lt2_ps = psg.tile([C, C], F32, tag="g2")
                    nc.tensor.transpose(lt2_ps[:], lts[:], ident[:])
                    nc.vector.tensor_copy(out=logits[:], in_=lt2_ps[:])
                    nc.scalar.activation(out=ee[:, s, :], in_=logits[:],
                                         func=mybir.ActivationFunctionType.Exp)
                    nc.vector.reduce_sum(
                        out=ssum[:, s, :],
                        in_=ee[:, s, :].rearrange("p (h d) -> p h d", h=HEADS),
                        axis=mybir.AxisListType.X)
                # rpn = 1/(Sp*Sn) per block
                rpn = smallp.tile([C, HEADS], F32, tag="rpn")
                nc.vector.tensor_mul(out=rpn[:], in0=ssum[:, 0, :],
                                     in1=ssum[:, 1, :])
                nc.vector.reciprocal(out=rpn[:], in_=rpn[:])
                scrh = smallp.tile([C, HEADS], F32, tag="scrh")
                rc1 = smallp.tile([C, 1], F32, tag="rc1")
                nc.vector.tensor_mul(out=scrh[:], in0=rpn[:], in1=hmask[:])
                nc.vector.reduce_sum(out=rc1[:], in_=scrh[:],
                                     axis=mybir.AxisListType.X)
                pp = smallp.tile([C, C], F32, tag="pp")
                nc.vector.tensor_mul(out=pp[:], in0=ee[:, 0, :],
                                     in1=ee[:, 1, :])
                nc.vector.tensor_scalar_mul(out=pp[:], in0=pp[:],
                                            scalar1=rc1[:])
                e2 = smallp.tile([C, C], F32, tag="e2")
                nc.scalar.activation(out=e2[:], in_=pp[:],
                                     func=mybir.ActivationFunctionType.Exp)
                s2 = smallp.tile([C, HEADS], F32, tag="s2")
                nc.vector.reduce_sum(
                    out=s2[:],
                    in_=e2[:].rearrange("p (h d) -> p h d", h=HEADS),
                    axis=mybir.AxisListType.X)
                nc.vector.reciprocal(out=s2[:], in_=s2[:])
                rc2 = smallp.tile([C, 1], F32, tag="rc2")
                nc.vector.tensor_mul(out=scrh[:], in0=s2[:], in1=hmask[:])
                nc.vector.reduce_sum(out=rc2[:], in_=scrh[:],
                                     axis=mybir.AxisListType.X)
                bd = smallp.tile([C, C], F32, tag="bd")
                nc.vector.tensor_scalar_mul(out=bd[:], in0=e2[:],
                                            scalar1=rc2[:])
                nc.vector.tensor_mul(out=bd[:], in0=bd[:], in1=bmask[:])
                mct_ps = psg.tile([C, C], F32, tag="g2")
                nc.tensor.matmul(mct_ps[:], lhsT=bd[:], rhs=wpt_sb[:],
                                 start=True, stop=True)
                nc.vector.tensor_copy(out=mct_sb[:, b, :], in_=mct_ps[:])

            def pass2_chunks(b):
                ops = [None] * NCH

                def mm(j):
                    t = psdw.tile([C, 2, W], F32, tag="dwps")
                    nc.tensor.matmul(
                        t[:], lhsT=mct_sb[:, b, :],
                        rhs=v_sum[:, b, 2 * W * j:2 * W * (j + 1)],
                        start=True, stop=True)
                    ops[j] = t

                def evac(j):
                    osb = outp.tile([C, 2, W], F32)
                    if j % 2 == 0:
                        nc.scalar.copy(out=osb[:], in_=ops[j][:])
                    else:
                        nc.vector.tensor_copy(out=osb[:], in_=ops[j][:])
                    nc.sync.dma_start(out=y[b, :, 2 * j:2 * j + 2, :],
                                      in_=osb[:])

                def step(j):
                    def go():
                        if j < NCH:
                            mm(j)
                        if j >= 1:
                            evac(j - 1)
                    return go
                return [step(j) for j in range(NCH + 1)]

            def pass2(b):
                for fn in pass2_chunks(b):
                    fn()

            # ------------ emission schedule ------------
            # b0 conv -> AR(b0) overlaps [v(b0), q(b1)] -> softmax(b0)
            # -> pass2(b0) after kn(b1) -> AR(b1) overlaps [v(b1), pass2(b0)]
            qk_unit(0, 0, xs["c", 0], True)
            qk_unit(0, 1, xs["p", 0], False)
            qk_unit(0, 2, xs["n", 0], False)
            stats_and_ar(0)
            v_unit(0)
            qk_unit(1, 0, xs["c", 1], True)
            softmax(0)
            qk_unit(1, 1, xs["p", 1], False)
            qk_unit(1, 2, xs["n", 1], False)
            stats_and_ar(1)
            v_unit(1, extras=pass2_chunks(0))
            softmax(1)
            pass2(1)

    nc.compile()
    return nc


def _prep_inputs(inputs):
    """Build per-core in_maps from full inputs."""
    x_curr = np.asarray(inputs["x_curr"], np.float32)
    x_prev = np.asarray(inputs["x_prev"], np.float32)
    x_next = np.asarray(inputs["x_next"], np.float32)
    w_q = np.asarray(inputs["w_q"], np.float32)
    w_q_dw = np.asarray(inputs["w_q_dw"], np.float32)
    w_kv_prev = np.asarray(inputs["w_kv_prev"], np.float32)
    w_kv_dw_prev = np.asarray(inputs["w_kv_dw_prev"], np.float32)
    w_kv_next = np.asarray(inputs["w_kv_next"], np.float32)
    w_kv_dw_next = np.asarray(inputs["w_kv_dw_next"], np.float32)
    w_proj = np.asarray(inputs["w_proj"], np.float32)
    temperature = np.asarray(inputs["temperature"], np.float32)

    # tap order implied by PAIR_T, with a zero pad tap in slot 9
    tap_order = []
    for t0, t1, _ in PAIR_T:
        tap_order.append(t0)
        tap_order.append(t1)

    def w3block(w1, wdw):
        # [C(in), 10 taps, C(out)], fp8, scaled by WSCALE
        wdw9 = wdw.reshape(C, 9)
        blk = np.zeros((C, 10, C), np.float32)
        for j, t in enumerate(tap_order):
            if t is None:
                continue
            blk[:, j, :] = WSCALE * np.einsum("oc,o->co", w1, wdw9[:, t])
        return blk

    w3qk = np.stack([
        w3block(w_q, w_q_dw),
        w3block(w_kv_prev[0:C], w_kv_dw_prev[0:C]),
        w3block(w_kv_next[0:C], w_kv_dw_next[0:C]),
    ], axis=1).astype(ml_dtypes.float8_e4m3)
    def w3nat(w1, wdw):
        return np.einsum("oc,ot->cto", w1, wdw.reshape(C, 9))

    w3v = np.stack([
        w3nat(w_kv_prev[C:2 * C], w_kv_dw_prev[C:2 * C]),
        w3nat(w_kv_next[C:2 * C], w_kv_dw_next[C:2 * C]),
    ], axis=1).astype(ml_dtypes.bfloat16)

    wpt = np.ascontiguousarray(w_proj.T)
    tmpv = np.repeat(temperature.reshape(HEADS), CH).reshape(C, 1)
    tmpv = np.ascontiguousarray(tmpv, np.float32)
    hmk = np.zeros((C, HEADS), np.float32)
    for h in range(HEADS):
        hmk[h * CH:(h + 1) * CH, h] = 1.0
    bmk = np.zeros((C, C), np.float32)
    for h in range(HEADS):
        bmk[h * CH:(h + 1) * CH, h * CH:(h + 1) * CH] = 1.0

    def strip(x, c, dt_):
        # guard-padded flat strip, quantized to dt_
        r0 = c * RPC - 1
        r1 = c * RPC + RPC + 1
        rows = np.zeros((B, C, SROWS, PITCH), np.float32)
        lo, hi = max(r0, 0), min(r1, H)
        rows[:, :, lo - r0:lo - r0 + hi - lo, 0:W] = x[:, :, lo:hi, :]
        out = np.zeros((B, C, XLEN), np.float32)
        out[:, :, LEAD:LEAD + SROWS * PITCH] = rows.reshape(B, C, -1)
        return out.astype(dt_)

    in_maps = []
    for c in range(N_CORES):
        f8, b16 = ml_dtypes.float8_e4m3, ml_dtypes.bfloat16
        in_maps.append({
            "x8c": strip(x_curr, c, f8),
            "x8p": strip(x_prev, c, f8),
            "x8n": strip(x_next, c, f8),
            "xbp": strip(x_prev, c, b16),
            "xbn": strip(x_next, c, b16),
            "w3qk": w3qk,
            "w3v": w3v,
            "wpt": wpt.astype(np.float32),
            "tmp": tmpv,
            "idn": np.eye(C, dtype=np.float32),
            "hmk": hmk,
            "bmk": bmk,
        })
    return in_maps


def kernel(**inputs):
    if "nc" not in _CACHE:
        _CACHE["nc"] = build_kernel()
    nc = _CACHE["nc"]
    in_maps = _prep_inputs(inputs)
    res = run_bass_kernel_spmd(nc, in_maps, core_ids=list(range(N_CORES)))
    out = np.empty((B, C, H, W), np.float32)
    for c in range(N_CORES):
        out[:, :, c * RPC:(c + 1) * RPC, :] = res.results[c]["y"]
    return out


if __name__ == "__main__":
    rng = np.random.default_rng(0)
    inputs = {
        "x_curr": rng.standard_normal((B, C, H, W), np.float32),
        "x_prev": rng.standard_normal((B, C, H, W), np.float32),
        "x_next": rng.standard_normal((B, C, H, W), np.float32),
        "w_q": rng.standard_normal((C, C), np.float32) * 0.02,
        "w_q_dw": rng.standard_normal((C, 1, 3, 3), np.float32) * 0.02,
        "w_kv_prev": rng.standard_normal((2 * C, C), np.float32) * 0.02,
        "w_kv_dw_prev": rng.standard_normal((2 * C, 1, 3, 3), np.float32) * 0.02,
        "w_kv_next": rng.standard_normal((2 * C, C), np.float32) * 0.02,
        "w_kv_dw_next": rng.standard_normal((2 * C, 1, 3, 3), np.float32) * 0.02,
        "w_proj": rng.standard_normal((C, C), np.float32) * 0.02,
        "temperature": np.ones((HEADS, 1, 1), np.float32),
    }
    out = kernel(**inputs)
    print("out", out.shape, out.dtype, np.abs(out).max())


# revision 23
# speedup vs baseline: 4.9211x; 1.0065x over previous
"""Co-Attention kernel for Trainium2, 8-core SPMD.

Sharding: spatial (H rows) across 8 cores; 32 rows/core with 1-row halo.
Per-core pipeline (all fused, single launch):
  - conv1x1+dwconv3x3 folded into shifted matmuls with combined weights
    W3_t[o,c] = W1[o,c] * wdw[o,t]; operands quantized to fp8e4 and run in
    DoubleRow perf mode with TWO taps packed per matmul via the k-tile dim
    (rhs dim1 = tap-pair with a strided/overlapping access pattern)
  - q/k: PSUM -> bf16 -> PE transpose -> [n,c] tiles -> Gram matrices
    (q@kT + self-Grams for the L2 norms) accumulated on PE over the
    core's spatial shard; v_prev+v_next accumulate into one PSUM chain
    and land in a bf16 SBUF-resident strip
  - per-batch AllReduce of the tiny Gram/norm stats across the 8 cores,
    software-pipelined against the other batch's conv pass
  - on-chip double softmax (block-diagonal channel attention)
  - output = (w_proj @ blockdiag(attn_co)) @ v_sum, one matmul per 2 rows
"""

import os
import sys

sys.path.insert(0, "/opt/trn_rl_repo")

import numpy as np
import ml_dtypes

import concourse.bacc as bacc
import concourse.bass as bass
import concourse.tile as tile
from concourse import mybir
from concourse.ap import AP
from concourse.bass_utils import run_bass_kernel_spmd

# problem constants
B, C, H, W = 2, 96, 256, 256
HEADS = 4
CH = C // HEADS
N_CORES = 8
RPC = H // N_CORES          # rows per core (32)
SROWS = RPC + 2             # strip rows incl halo (34)
PITCH = W + 2               # guarded row pitch (258)
LEAD = 2                    # leading guard pad
XLEN = LEAD + SROWS * PITCH + 2  # strip flat length (8776)
NCH = RPC // 2              # 2-row chunks per core (16)
NTILES = RPC * 2            # 128-wide transpose tiles per unit per b (64)
VLEN = RPC * W              # v_sum flat length per b (8192)
WSCALE = 4096.0             # fp8 weight scale (exact power of 2)

F32 = mybir.dt.float32
BF16 = mybir.dt.bfloat16
F8 = mybir.dt.float8e4
DR = mybir.MatmulPerfMode.DoubleRow

# tap offsets (cross-correlation, matching jax.lax.conv_general_dilated)
TAPS = [(ky - 1) * PITCH + (kx - 1) for ky in range(3) for kx in range(3)]
# tap-pair packing for DoubleRow: (first tap, second tap, rhs dim1 stride);
# last pair duplicates tap 8 with zero weights (stride 0)
PAIR_T = [(0, 1, 1), (2, 5, PITCH), (3, 4, 1), (6, 7, 1), (8, None, 0)]

_CACHE = {}


def rowoff(r):
    return LEAD + r * PITCH


def pair_rhs(xt, j, m):
    """Overlapping strided rhs AP [C, ktile=2, rows=2, cols=256] for the
    DoubleRow tap-pair matmul of chunk j, pair m."""
    t0, _, dstride = PAIR_T[m]
    base = xt[:]
    off = base.offset + rowoff(1 + 2 * j) + TAPS[t0]
    ppair = list(base.ap[0])
    return AP(base.tensor, off,
              [ppair, [dstride, 2], [PITCH, 2], [1, W]])


def row2_rhs(xt, j, t):
    """Plain 2-row rhs AP [C, rows=2, cols=256] for tap t of chunk j."""
    base = xt[:]
    off = base.offset + rowoff(1 + 2 * j) + TAPS[t]
    ppair = list(base.ap[0])
    return AP(base.tensor, off, [ppair, [PITCH, 2], [1, W]])


def build_kernel():
    SKIP_AR = bool(os.environ.get("SKIP_AR"))
    nc = bacc.Bacc("TRN2", target_bir_lowering=False, debug=False,
                   num_devices=N_CORES)

    x8c = nc.declare_dram_parameter("x8c", [B, C, XLEN], F8, isOutput=False)
    x8p = nc.declare_dram_parameter("x8p", [B, C, XLEN], F8, isOutput=False)
    x8n = nc.declare_dram_parameter("x8n", [B, C, XLEN], F8, isOutput=False)
    w3qk = nc.declare_dram_parameter("w3qk", [C, 3, 10, C], F8, isOutput=False)
    xbp = nc.declare_dram_parameter("xbp", [B, C, XLEN], BF16, isOutput=False)
    xbn = nc.declare_dram_parameter("xbn", [B, C, XLEN], BF16, isOutput=False)
    w3v = nc.declare_dram_parameter("w3v", [C, 2, 9, C], BF16, isOutput=False)
    wpt = nc.declare_dram_parameter("wpt", [C, C], F32, isOutput=False)
    tmp = nc.declare_dram_parameter("tmp", [C, 1], F32, isOutput=False)
    idn = nc.declare_dram_parameter("idn", [C, C], F32, isOutput=False)
    hmk = nc.declare_dram_parameter("hmk", [C, HEADS], F32, isOutput=False)
    bmk = nc.declare_dram_parameter("bmk", [C, C], F32, isOutput=False)
    y = nc.declare_dram_parameter("y", [B, C, RPC, W], F32, isOutput=True)

    ar_in = [nc.dram_tensor(f"ar_in{b}", [C, 195], F32) for b in range(B)]
    ar_out = [nc.dram_tensor(f"ar_out{b}", [C, 195], F32, addr_space="Shared")
              for b in range(B)]

    with tile.TileContext(nc) as tc:
        with (
            tc.tile_pool(name="singles", bufs=1) as singles,
            tc.tile_pool(name="xpool", bufs=1) as xpool,
            tc.tile_pool(name="dwsb", bufs=3) as dwsbp,
            tc.tile_pool(name="qstorep", bufs=2) as qstorep,
            tc.tile_pool(name="kstore", bufs=3) as kstorep,
            tc.tile_pool(name="small", bufs=1) as smallp,
            tc.tile_pool(name="outp", bufs=3) as outp,
            tc.tile_pool(name="psdw", bufs=4, space="PSUM") as psdw,
            tc.tile_pool(name="pstp", bufs=2, space="PSUM") as pstp,
            tc.tile_pool(name="psg", bufs=1, space="PSUM") as psg,
        ):
            # ---- constants ----
            w3qk_sb = singles.tile([C, 3, 10, C], F8)
            nc.sync.dma_start(out=w3qk_sb[:], in_=w3qk[:, :, :, :])
            w3v_sb = singles.tile([C, 2, 9, C], BF16)
            nc.sync.dma_start(out=w3v_sb[:], in_=w3v[:, :, :, :])
            wpt_sb = singles.tile([C, C], F32)
            nc.sync.dma_start(out=wpt_sb[:], in_=wpt[:, :])
            temp_sb = singles.tile([C, 1], F32)
            nc.sync.dma_start(out=temp_sb[:], in_=tmp[:, :])
            ident = singles.tile([C, C], F32)
            nc.sync.dma_start(out=ident[:], in_=idn[:, :])
            identb = singles.tile([C, C], BF16)
            nc.vector.tensor_copy(out=identb[:], in_=ident[:])
            hmask = singles.tile([C, HEADS], F32)
            nc.sync.dma_start(out=hmask[:], in_=hmk[:, :])
            bmask = singles.tile([C, C], F32)
            nc.sync.dma_start(out=bmask[:], in_=bmk[:, :])

            # input strips: all 6 up-front (no deps, fills SBUF early)
            xs = {}
            for b in range(B):
                for key, src in (("c", x8c), ("p", x8p), ("n", x8n)):
                    pass
                for key, src in (("c", x8c), ("p", x8p), ("n", x8n),
                                 ("bp", xbp), ("bn", xbn)):
                    dt_ = BF16 if key.startswith("b") else F8
                    t = xpool.tile([C, XLEN], dt_, tag=f"x{key}{b}",
                                   name="xstrip")
                    if b == 0 and key == "c":
                        cut = rowoff(10)
                        nc.sync.dma_start(out=t[:, 0:cut],
                                          in_=src[b][:, 0:cut])
                        nc.sync.dma_start(out=t[:, cut:XLEN],
                                          in_=src[b][:, cut:XLEN])
                    else:
                        nc.sync.dma_start(out=t[:], in_=src[b])
                    xs[key, b] = t

            # persistent accumulators
            v_sum = singles.tile([C, B, VLEN], BF16)
            ar_sb = singles.tile([C, B, 195], F32)
            gram_sb = singles.tile([C, B, 5, C], F32)
            arr_sb = singles.tile([C, B, 195], F32)
            mct_sb = singles.tile([C, B, C], BF16)

            qstore = {}

            # ------------ pass 1 pieces ------------
            def qk_unit(b, u, xt, is_q):
                """q or k unit: 5-stage chunk pipeline.
                stages: taps(PE) -> evac(Act) -> transpose(PE) ->
                        ucopy(DVE) -> grams(PE)."""
                if is_q:
                    qstore[b] = qstorep.tile([128, NTILES, C], BF16,
                                             tag="qstore", name="qstore")
                    g_self = psg.tile([C, C], F32, tag="g")
                    g_cross = None
                else:
                    g_self = psg.tile([C, C], F32, tag="g")
                    g_cross = psg.tile([C, C], F32, tag="g2")
                dw = [None] * NCH
                sb = [None] * NCH
                tpt = [None] * NCH
                kt = [None] * NCH

                def taps(j):
                    dwps = psdw.tile([C, 2, W], F32, tag="dwps")
                    for m in range(5):
                        nc.tensor.matmul(
                            dwps[:], lhsT=w3qk_sb[:, u, 2 * m:2 * m + 2, :],
                            rhs=pair_rhs(xt, j, m),
                            start=(m == 0), stop=(m == 4), perf_mode=DR)
                    dw[j] = dwps

                def evac(j):
                    dwsb = dwsbp.tile([C, 2, W], BF16)
                    nc.scalar.copy(out=dwsb[:], in_=dw[j][:])
                    sb[j] = dwsb

                def transp(j):
                    tp = pstp.tile([128, 4, C], BF16, tag="tp")
                    for r2 in range(2):
                        for hf in range(2):
                            nc.tensor.transpose(
                                tp[:, 2 * r2 + hf, :],
                                sb[j][:, r2, 128 * hf:128 * hf + 128],
                                identb[:])
                    tpt[j] = tp

                def ucopy(j):
                    if is_q:
                        dst = qstore[b][:, 4 * j:4 * j + 4, :]
                    else:
                        kt[j] = kstorep.tile([128, 4, C], BF16, tag="kT",
                                             name="kt")
                        dst = kt[j][:]
                    nc.vector.tensor_copy(out=dst, in_=tpt[j][:])

                def gram(j):
                    for i in range(4):
                        gi = 4 * j + i
                        st = (gi == 0)
                        sp = (gi == NTILES - 1)
                        if is_q:
                            nc.tensor.matmul(
                                g_self[:], lhsT=qstore[b][:, gi, :],
                                rhs=qstore[b][:, gi, :], start=st, stop=sp,
                                skip_group_check=True)
                        else:
                            nc.tensor.matmul(
                                g_cross[:], lhsT=qstore[b][:, gi, :],
                                rhs=kt[j][:, i, :], start=st, stop=sp,
                                skip_group_check=True)
                            nc.tensor.matmul(
                                g_self[:], lhsT=kt[j][:, i, :],
                                rhs=kt[j][:, i, :], start=st, stop=sp,
                                skip_group_check=True)

                stages = [taps, evac, transp, ucopy, gram]
                for j in range(NCH + 4):
                    for s, fn in enumerate(stages):
                        if 0 <= j - s < NCH:
                            fn(j - s)

                # evacuate gram psums
                if is_q:
                    nc.vector.tensor_copy(out=gram_sb[:, b, 0, :],
                                          in_=g_self[:])
                else:
                    slot = 1 if u == 1 else 3
                    nc.vector.tensor_copy(out=gram_sb[:, b, slot, :],
                                          in_=g_cross[:])
                    nc.vector.tensor_copy(out=gram_sb[:, b, slot + 1, :],
                                          in_=g_self[:])

            def v_unit(b, extras=None):
                """fused v_prev+v_next: 10 DR tap matmuls into one PSUM
                accumulation per chunk, Pool evacuates to bf16 v_sum."""
                dw = [None] * NCH

                def taps(j):
                    dwps = psdw.tile([C, 2, W], F32, tag="dwps")
                    for part, key in enumerate(("bp", "bn")):
                        for t in range(9):
                            nc.tensor.matmul(
                                dwps[:],
                                lhsT=w3v_sb[:, part, t, :],
                                rhs=row2_rhs(xs[key, b], j, t),
                                start=(part == 0 and t == 0),
                                stop=(part == 1 and t == 8))
                    dw[j] = dwps

                def evac(j):
                    nc.vector.tensor_copy(
                        out=v_sum[:, b, 2 * W * j:2 * W * (j + 1)].rearrange(
                            "p (r w) -> p r w", w=W),
                        in_=dw[j][:])

                for j in range(NCH + 1):
                    if j < NCH:
                        taps(j)
                    if j >= 1:
                        evac(j - 1)
                    if extras and j < len(extras):
                        extras[j]()

            def stats_and_ar(b):
                # diag extraction via masked reduce + AllReduce kickoff
                scr = smallp.tile([C, C], F32, tag="scr")
                for k, slot in enumerate((0, 2, 4)):
                    nc.vector.tensor_mul(out=scr[:],
                                         in0=gram_sb[:, b, slot, :],
                                         in1=ident[:])
                    nc.vector.reduce_sum(out=ar_sb[:, b, 192 + k:193 + k],
                                         in_=scr[:],
                                         axis=mybir.AxisListType.X)
                nc.vector.tensor_copy(out=ar_sb[:, b, 0:96],
                                      in_=gram_sb[:, b, 1, :])
                nc.vector.tensor_copy(out=ar_sb[:, b, 96:192],
                                      in_=gram_sb[:, b, 3, :])
                if SKIP_AR:
                    nc.vector.tensor_copy(out=arr_sb[:, b, :],
                                          in_=ar_sb[:, b, :])
                else:
                    nc.sync.dma_start(out=ar_in[b][:, :], in_=ar_sb[:, b, :])
                    nc.gpsimd.collective_compute(
                        "AllReduce", mybir.AluOpType.add,
                        replica_groups=[list(range(N_CORES))],
                        ins=[ar_in[b][:, :]], outs=[ar_out[b][:, :]],
                    )
                    nc.sync.dma_start(out=arr_sb[:, b, :],
                                      in_=ar_out[b][:, :])

            def softmax(b):
                rinv = smallp.tile([C, 3], F32, tag="rinv")
                nc.scalar.activation(out=rinv[:], in_=arr_sb[:, b, 192:195],
                                     func=mybir.ActivationFunctionType.Sqrt)
                nc.vector.tensor_scalar_max(out=rinv[:], in0=rinv[:],
                                            scalar1=1e-12)
                nc.vector.reciprocal(out=rinv[:], in_=rinv[:])
                rqt = smallp.tile([C, 1], F32, tag="rqt")
                nc.vector.tensor_mul(out=rqt[:], in0=rinv[:, 0:1],
                                     in1=temp_sb[:])

                ee = smallp.tile([C, 2, C], F32, tag="ee")
                ssum = smallp.tile([C, 2, HEADS], F32, tag="ssum")
                for s in range(2):
                    logits = smallp.tile([C, C], F32, tag="logits")
                    nc.vector.tensor_scalar_mul(
                        out=logits[:], in0=arr_sb[:, b, 96 * s:96 * s + 96],
                        scalar1=rqt[:])
                    # column scale via transpose sandwich
                    lt_ps = psg.tile([C, C], F32, tag="g")
                    nc.tensor.transpose(lt_ps[:], logits[:], ident[:])
                    lts = smallp.tile([C, C], F32, tag="lts")
                    nc.vector.tensor_scalar_mul(out=lts[:], in0=lt_ps[:],
                                                scalar1=rinv[:, 1 + s:2 + s])
                    lt2_ps = psg.tile([C, C], F32, tag="g2")
                    nc.tensor.transpose(lt2_ps[:], lts[:], ident[:])
                    nc.vector.tensor_copy(out=logits[:], in_=lt2_ps[:])
                    nc.scalar.activation(out=ee[:, s, :], in_=logits[:],
                                         func=mybir.ActivationFunctionType.Exp)
                    nc.vector.reduce_sum(
                        out=ssum[:, s, :],
                        in_=ee[:, s, :].rearrange("p (h d) -> p h d", h=HEADS),
                        axis=mybir.AxisListType.X)
                # rpn = 1/(Sp*Sn) per block
                rpn = smallp.tile([C, HEADS], F32, tag="rpn")
                nc.vector.tensor_mul(out=rpn[:], in0=ssum[:, 0, :],
                                     in1=ssum[:, 1, :])
                nc.vector.reciprocal(out=rpn[:], in_=rpn[:])
                scrh = smallp.tile([C, HEADS], F32, tag="scrh")
                rc1 = smallp.tile([C, 1], F32, tag="rc1")
                nc.vector.tensor_mul(out=scrh[:], in0=rpn[:], in1=hmask[:])
                nc.vector.reduce_sum(out=rc1[:], in_=scrh[:],
                                     axis=mybir.AxisListType.X)
                pp = smallp.tile([C, C], F32, tag="pp")
                nc.vector.tensor_mul(out=pp[:], in0=ee[:, 0, :],
                                     in1=ee[:, 1, :])
                nc.vector.tensor_scalar_mul(out=pp[:], in0=pp[:],
                                            scalar1=rc1[:])
                e2 = smallp.tile([C, C], F32, tag="e2")
                nc.scalar.activation(out=e2[:], in_=pp[:],
                                     func=mybir.ActivationFunctionType.Exp)
                s2 = smallp.tile([C, HEADS], F32, tag="s2")
                nc.vector.reduce_sum(
                    out=s2[:],
                    in_=e2[:].rearrange("p (h d) -> p h d", h=HEADS),
                    axis=mybir.AxisListType.X)
                nc.vector.reciprocal(out=s2[:], in_=s2[:])
                rc2 = smallp.tile([C, 1], F32, tag="rc2")
                nc.vector.tensor_mul(out=scrh[:], in0=s2[:], in1=hmask[:])
                nc.vector.reduce_sum(out=rc2[:], in_=scrh[:],
                                     axis=mybir.AxisListType.X)
                bd = smallp.tile([C, C], F32, tag="bd")
                nc.vector.tensor_scalar_mul(out=bd[:], in0=e2[:],
                                            scalar1=rc2[:])
                nc.vector.tensor_mul(out=bd[:], in0=bd[:], in1=bmask[:])
                mct_ps = psg.tile([C, C], F32, tag="g2")
                nc.tensor.matmul(mct_ps[:], lhsT=bd[:], rhs=wpt_sb[:],
                                 start=True, stop=True)
                nc.vector.tensor_copy(out=mct_sb[:, b, :], in_=mct_ps[:])

            def pass2_chunks(b):
                ops = [None] * NCH

                def mm(j):
                    t = psdw.tile([C, 2, W], F32, tag="dwps")
                    nc.tensor.matmul(
                        t[:], lhsT=mct_sb[:, b, :],
                        rhs=v_sum[:, b, 2 * W * j:2 * W * (j + 1)],
                        start=True, stop=True)
                    ops[j] = t

                def evac(j):
                    osb = outp.tile([C, 2, W], F32)
                    if j % 2 == 0:
                        nc.scalar.copy(out=osb[:], in_=ops[j][:])
                    else:
                        nc.vector.tensor_copy(out=osb[:], in_=ops[j][:])
                    nc.sync.dma_start(out=y[b, :, 2 * j:2 * j + 2, :],
                                      in_=osb[:])

                def step(j):
                    def go():
                        if j < NCH:
                            mm(j)
                        if j >= 1:
                            evac(j - 1)
                    return go
                return [step(j) for j in range(NCH + 1)]

            def pass2(b):
                for fn in pass2_chunks(b):
                    fn()

            # ------------ emission schedule ------------
            # b0 conv -> AR(b0) overlaps [v(b0), q(b1)] -> softmax(b0)
            # -> pass2(b0) after kn(b1) -> AR(b1) overlaps [v(b1), pass2(b0)]
            qk_unit(0, 0, xs["c", 0], True)
            qk_unit(0, 1, xs["p", 0], False)
            qk_unit(0, 2, xs["n", 0], False)
            stats_and_ar(0)
            v_unit(0)
            qk_unit(1, 0, xs["c", 1], True)
            softmax(0)
            qk_unit(1, 1, xs["p", 1], False)
            qk_unit(1, 2, xs["n", 1], False)
            stats_and_ar(1)
            v_unit(1, extras=pass2_chunks(0))
            softmax(1)
            pass2(1)

    nc.compile()
    return nc


def _prep_inputs(inputs):
    """Build per-core in_maps from full inputs."""
    x_curr = np.asarray(inputs["x_curr"], np.float32)
    x_prev = np.asarray(inputs["x_prev"], np.float32)
    x_next = np.asarray(inputs["x_next"], np.float32)
    w_q = np.asarray(inputs["w_q"], np.float32)
    w_q_dw = np.asarray(inputs["w_q_dw"], np.float32)
    w_kv_prev = np.asarray(inputs["w_kv_prev"], np.float32)
    w_kv_dw_prev = np.asarray(inputs["w_kv_dw_prev"], np.float32)
    w_kv_next = np.asarray(inputs["w_kv_next"], np.float32)
    w_kv_dw_next = np.asarray(inputs["w_kv_dw_next"], np.float32)
    w_proj = np.asarray(inputs["w_proj"], np.float32)
    temperature = np.asarray(inputs["temperature"], np.float32)

    # tap order implied by PAIR_T, with a zero pad tap in slot 9
    tap_order = []
    for t0, t1, _ in PAIR_T:
        tap_order.append(t0)
        tap_order.append(t1)

    def w3block(w1, wdw):
        # [C(in), 10 taps, C(out)], fp8, scaled by WSCALE
        wdw9 = wdw.reshape(C, 9)
        blk = np.zeros((C, 10, C), np.float32)
        for j, t in enumerate(tap_order):
            if t is None:
                continue
            blk[:, j, :] = WSCALE * np.einsum("oc,o->co", w1, wdw9[:, t])
        return blk

    w3qk = np.stack([
        w3block(w_q, w_q_dw),
        w3block(w_kv_prev[0:C], w_kv_dw_prev[0:C]),
        w3block(w_kv_next[0:C], w_kv_dw_next[0:C]),
    ], axis=1).astype(ml_dtypes.float8_e4m3)
    def w3nat(w1, wdw):
        return np.einsum("oc,ot->cto", w1, wdw.reshape(C, 9))

    w3v = np.stack([
        w3nat(w_kv_prev[C:2 * C], w_kv_dw_prev[C:2 * C]),
        w3nat(w_kv_next[C:2 * C], w_kv_dw_next[C:2 * C]),
    ], axis=1).astype(ml_dtypes.bfloat16)

    wpt = np.ascontiguousarray(w_proj.T)
    tmpv = np.repeat(temperature.reshape(HEADS), CH).reshape(C, 1)
    tmpv = np.ascontiguousarray(tmpv, np.float32)
    hmk = np.zeros((C, HEADS), np.float32)
    for h in range(HEADS):
        hmk[h * CH:(h + 1) * CH, h] = 1.0
    bmk = np.zeros((C, C), np.float32)
    for h in range(HEADS):
        bmk[h * CH:(h + 1) * CH, h * CH:(h + 1) * CH] = 1.0

    def strip(x, c, dt_):
        # guard-padded flat strip, quantized to dt_
        r0 = c * RPC - 1
        r1 = c * RPC + RPC + 1
        rows = np.zeros((B, C, SROWS, PITCH), np.float32)
        lo, hi = max(r0, 0), min(r1, H)
        rows[:, :, lo - r0:lo - r0 + hi - lo, 0:W] = x[:, :, lo:hi, :]
        out = np.zeros((B, C, XLEN), np.float32)
        out[:, :, LEAD:LEAD + SROWS * PITCH] = rows.reshape(B, C, -1)
        return out.astype(dt_)

    in_maps = []
    for c in range(N_CORES):
        f8, b16 = ml_dtypes.float8_e4m3, ml_dtypes.bfloat16
        in_maps.append({
            "x8c": strip(x_curr, c, f8),
            "x8p": strip(x_prev, c, f8),
            "x8n": strip(x_next, c, f8),
            "xbp": strip(x_prev, c, b16),
            "xbn": strip(x_next, c, b16),
            "w3qk": w3qk,
            "w3v": w3v,
            "wpt": wpt.astype(np.float32),
            "tmp": tmpv,
            "idn": np.eye(C, dtype=np.float32),
            "hmk": hmk,
            "bmk": bmk,
        })
    return in_maps


def kernel(**inputs):
    if "nc" not in _CACHE:
        _CACHE["nc"] = build_kernel()
    nc = _CACHE["nc"]
    in_maps = _prep_inputs(inputs)
    res = run_bass_kernel_spmd(nc, in_maps, core_ids=list(range(N_CORES)))
    out = np.empty((B, C, H, W), np.float32)
    for c in range(N_CORES):
        out[:, :, c * RPC:(c + 1) * RPC, :] = res.results[c]["y"]
    return out


if __name__ == "__main__":
    rng = np.random.default_rng(0)
    inputs = {
        "x_curr": rng.standard_normal((B, C, H, W), np.float32),
        "x_prev": rng.standard_normal((B, C, H, W), np.float32),
        "x_next": rng.standard_normal((B, C, H, W), np.float32),
        "w_q": rng.standard_normal((C, C), np.float32) * 0.02,
        "w_q_dw": rng.standard_normal((C, 1, 3, 3), np.float32) * 0.02,
        "w_kv_prev": rng.standard_normal((2 * C, C), np.float32) * 0.02,
        "w_kv_dw_prev": rng.standard_normal((2 * C, 1, 3, 3), np.float32) * 0.02,
        "w_kv_next": rng.standard_normal((2 * C, C), np.float32) * 0.02,
        "w_kv_dw_next": rng.standard_normal((2 * C, 1, 3, 3), np.float32) * 0.02,
        "w_proj": rng.standard_normal((C, C), np.float32) * 0.02,
        "temperature": np.ones((HEADS, 1, 1), np.float32),
    }
    out = kernel(**inputs)
    print("out", out.shape, out.dtype, np.abs(out).max())
